# revision 1
# baseline (speedup 1.0000x reference)
"""Multi-head attention (RoPE) Trainium2 Bass kernel.

Problem: B=4, T=2048, C=1024, H=16, d=64, fp32, full attention + RoPE.
Sharding: 8 cores = 4 batches x 2 head-groups (8 heads each). Each core
computes its batch's attention for its heads plus the partial output
projection; the host sums the two head-group partials per batch.

All matmuls run as float32r (full fp32 precision, 1 cycle/row at free>=256).
"""

import numpy as np

B, T, C = 4, 2048, 1024
H, D = 16, 64
G = 2              # head groups (cores per batch)
HG = H // G        # heads per core = 8
JQK = C // G // 128  # 4 q-chunks + 4 k-chunks of 128 features
CC = C // 128      # 8 contraction chunks
NTB = T // 512     # 4 t-blocks for qkv phase
NKC = T // 128     # 16 key chunks
ROPE_BASE = 10000.0
SCALE = 1.0 / np.sqrt(D)

_CACHED = {}


def _rope_tables():
    inv_freq = 1.0 / (ROPE_BASE ** (np.arange(0, D, 2, dtype=np.float32) / D))
    t = np.arange(T, dtype=np.float32)
    freqs = np.outer(t, inv_freq).astype(np.float32)          # (T, 32)
    emb = np.concatenate([freqs, freqs], axis=-1)             # (T, 64)
    cos = np.cos(emb).T.astype(np.float32)                    # (64, T)
    sin = np.sin(emb).T.astype(np.float32)                    # (64, T)
    cosT = np.concatenate([cos, cos], axis=0)                 # (128, T) two heads per chunk
    sinT = np.concatenate([sin, sin], axis=0)                 # (128, T)
    return np.ascontiguousarray(cosT), np.ascontiguousarray(sinT)


def _round_fp32r(a):
    """Round fp32 array to fp32r precision (11-bit mantissa, RNE)."""
    u = np.ascontiguousarray(a, np.float32).view(np.uint32).copy()
    lsb = (u >> 12) & 1
    u += 0x7FF + lsb
    u &= np.uint32(0xFFFFF000)
    return u.view(np.float32)


def _perm_tables():
    # rope_perm: rot[d] = sum_s P[s, d] * raw[s] = rotate_half with sign
    P = np.zeros((128, 128), np.float32)
    for d in range(128):
        blk, dd = divmod(d, D)
        if dd < 32:
            P[blk * D + dd + 32, d] = -1.0
        else:
            P[blk * D + dd - 32, d] = 1.0
    # shift64: out[d] = nrm[d - 64] for d >= 64 (move partitions 0:64 -> 64:128)
    S = np.zeros((64, 128), np.float32)
    for d in range(64, 128):
        S[d - 64, d] = 1.0
    # iden64: out[d] = nrm[d] for d < 64 (keep partitions 0:64)
    I = np.zeros((64, 128), np.float32)
    for d in range(64):
        I[d, d] = 1.0
    return P, S, I


def _attn_body(tc, outs, ins):
    """Tile kernel body. ins/outs are dicts of DRAM APs."""
    import contextlib
    import concourse.mybir as mybir

    from concourse.tile import add_dep_helper

    nc = tc.nc
    F32 = mybir.dt.float32
    F32R = mybir.dt.float32r
    EXP = mybir.ActivationFunctionType.Exp

    xT = ins["xT"]            # (1024, 2048)  x[b].T
    wqkv = ins["wqkv"]        # (1024, 1536)  [Wq | Wk | Wv] cols for this group
    wproj = ins["wproj"]      # (512, 1024)
    bqk = ins["bqk"]          # (128, 8) per-chunk per-partition bias
    bv = ins["bv"]            # (128, 512) broadcast v bias
    bproj = ins["bproj"]      # (128, 1024) broadcast proj bias (zeros on group-1 cores)
    cosT_d = ins["cosT"]      # (128, 2048)
    sinT_d = ins["sinT"]      # (128, 2048)
    perm_d = ins["rope_perm"]  # (128, 128) signed rotate_half permutation
    shift_d = ins["shift64"]  # (64, 128) partition up-shift matrix
    iden_d = ins["iden64"]    # (64, 128) partition keep matrix
    out = outs["out"]         # (2048, 1024) partial output

    ctx = contextlib.ExitStack()
    with ctx:
        pers = ctx.enter_context(tc.tile_pool(name="pers", bufs=1))

        # persistent tiles
        qk = [pers.tile([128, T], F32R, name=f"qk{j}", tag=f"qk{j}") for j in range(8)]
        vg = [pers.tile([128, HG * 65], F32R, name=f"vg{k}", tag=f"vg{k}") for k in range(NKC)]
        cos_t = pers.tile([128, T], F32, name="cos_t", tag="cos_t")
        sin_t = pers.tile([128, T], F32, name="sin_t", tag="sin_t")
        bqk_t = pers.tile([128, 8], F32, name="bqk_t", tag="bqk_t")
        bv_t = pers.tile([128, HG * 65], F32, name="bv_t", tag="bv_t")
        bproj_t = pers.tile([128, 1024], F32, name="bproj_t", tag="bproj_t")
        perm_t = pers.tile([128, 128], F32R, name="perm_t", tag="perm_t")
        shift_t = pers.tile([64, 128], F32R, name="shift_t", tag="shift_t")
        iden_t = pers.tile([64, 128], F32R, name="iden_t", tag="iden_t")

        # chunk big constant loads so each consumer slice waits on one DMA
        for tb in range(NTB):
            tsl = slice(tb * 512, (tb + 1) * 512)
            nc.sync.dma_start(cos_t[:, tsl], cosT_d[:, tsl])
            nc.sync.dma_start(sin_t[:, tsl], sinT_d[:, tsl])
        nc.sync.dma_start(bqk_t, bqk)
        nc.sync.dma_start(bv_t, bv)
        for ec in range(2):
            esl = slice(ec * 512, (ec + 1) * 512)
            nc.sync.dma_start(bproj_t[:, esl], bproj[:, esl])
        d_perm = nc.sync.dma_start(perm_t, perm_d)
        d_shift = nc.sync.dma_start(shift_t, shift_d)
        d_iden = nc.sync.dma_start(iden_t, iden_d)

        def dbg(name, tile_ap):
            if name in outs:
                nc.sync.dma_start(outs[name].bitcast(tile_ap.dtype), tile_ap)

        # ---------------- Phase 1: QKV projection + RoPE ----------------
        with contextlib.ExitStack() as p1:
            wpool = p1.enter_context(tc.tile_pool(name="wpool", bufs=1))
            xpool = p1.enter_context(tc.tile_pool(name="xpool", bufs=10))
            scratch = p1.enter_context(tc.tile_pool(name="scratch", bufs=3))
            psQ = p1.enter_context(tc.tile_pool(name="psQ", bufs=5, space="PSUM"))

            w_t = [wpool.tile([128, 1536], F32R, name=f"w{c}", tag=f"w{c}") for c in range(CC)]
            w_dmas = [d_perm, d_shift, d_iden]
            for c in range(CC):
                for jc in range(12):
                    jsl = slice(jc * 128, (jc + 1) * 128)
                    w_dmas.append(nc.sync.dma_start(
                        w_t[c][:, jsl], wqkv[c * 128:(c + 1) * 128, jsl]))

            for tb in range(NTB):
                tsl = slice(tb * 512, (tb + 1) * 512)
                # stream xT chunk tiles for this t-block
                xt = []
                x_dmas = list(w_dmas) if tb == 0 else []
                for c in range(CC):
                    xtile = xpool.tile([128, 512], F32R, name=f"xt{tb}_{c}", tag="xt")
                    x_dmas.append(nc.sync.dma_start(xtile, xT[c * 128:(c + 1) * 128, tsl]))
                    xt.append(xtile)

                # q,k chunks -> transposed layout (feature on partitions)
                for jc in range(8):
                    ps = psQ.tile([128, 512], F32, name=f"psqk{tb}_{jc}", tag="ps")
                    for c in range(CC):
                        nc.tensor.matmul(
                            ps,
                            w_t[c][:, jc * 128:(jc + 1) * 128],
                            xt[c],
                            start=(c == 0), stop=(c == CC - 1),
                        )
                    # bias add (per-partition) into raw tile
                    raw = scratch.tile([128, 512], F32R, name=f"raw{tb}_{jc}", tag="raw")
                    nc.vector.tensor_scalar_add(raw, ps, bqk_t[:, jc:jc + 1])
                    # RoPE: qk' = raw*cos + (perm @ raw)*sin; perm carries the
                    # rotate_half partition shuffle + sign (PE, partition-safe)
                    rps = psQ.tile([128, 512], F32, name=f"rps{tb}_{jc}", tag="ps")
                    nc.tensor.matmul(rps, perm_t, raw, start=True, stop=True)
                    tmp = scratch.tile([128, 512], F32, name=f"tmp{tb}_{jc}", tag="tmp")
                    nc.vector.tensor_mul(tmp, rps, sin_t[:, tsl])
                    tmpc = scratch.tile([128, 512], F32, name=f"tmpc{tb}_{jc}", tag="tmpc")
                    nc.vector.tensor_mul(tmpc, raw, cos_t[:, tsl])
                    nc.vector.tensor_add(qk[jc][:, tsl], tmpc, tmp)

                # v -> token-on-partition layout with ones column, v = x @ Wv + bv
                for tsub in range(4):
                    kc = tb * 4 + tsub
                    psv = psQ.tile([128, 512], F32, name=f"psv{kc}", tag="ps")
                    for c in range(CC):
                        nc.tensor.matmul(
                            psv,
                            xt[c][:, tsub * 128:(tsub + 1) * 128],
                            w_t[c][:, 1024:1536],
                            start=(c == 0), stop=(c == CC - 1),
                        )
                    vv = vg[kc].rearrange("p (g d) -> p g d", g=HG)
                    bvv = bv_t.rearrange("p (g d) -> p g d", g=HG)
                    nc.vector.tensor_add(
                        vv[:, :, 0:64],
                        psv.rearrange("p (g d) -> p g d", g=HG),
                        bvv[:, :, 0:64],
                    )
                    nc.vector.tensor_copy(vv[:, :, 64:65], bvv[:, :, 64:65])

        dbg("dbg_qk0", qk[0])
        dbg("dbg_qk4", qk[4])
        dbg("dbg_vg0", vg[0])

        # ---------------- Phase 2: attention per head ----------------
        with contextlib.ExitStack() as p2:
            aT = []
            atp = tc.tile_pool(name="atp", bufs=1)
            p2a = p2.enter_context(atp)
            for i in range(4):
                aT.append(p2a.tile([128, T], F32R, name=f"aT{i}", tag=f"aT{i}"))
            wppool = p2.enter_context(tc.tile_pool(name="wppool", bufs=1))
            wp_t = [wppool.tile([128, 1024], F32R, name=f"wp{i}", tag=f"wp{i}") for i in range(4)]
            wp_dmas = []
            for i in range(4):
                for ec in range(2):
                    esl = slice(ec * 512, (ec + 1) * 512)
                    wp_dmas.append(nc.sync.dma_start(
                        wp_t[i][:, esl], wproj[i * 128:(i + 1) * 128, esl]))

            with contextlib.ExitStack() as p2b:
                import concourse.bass as bass
                drp = p2b.enter_context(tc.tile_pool(name="drp", bufs=2, space="DRAM"))
                expp = p2b.enter_context(tc.tile_pool(name="expp", bufs=3))
                bcp = p2b.enter_context(tc.tile_pool(name="bcp", bufs=2))
                rcpp = p2b.enter_context(tc.tile_pool(name="rcpp", bufs=1))
                psS = p2b.enter_context(tc.tile_pool(name="psS", bufs=2, space="PSUM"))
                psAV = p2b.enter_context(tc.tile_pool(name="psAV", bufs=2, space="PSUM"))

                for h in range(HG):
                    hc = h // 2            # 128-chunk index for this head
                    ho = (h % 2) * 64      # row offset within chunk
                    qt = qk[hc]
                    kt = qk[4 + hc]
                    for qh in range(2):    # 1024-wide q halves
                        q0 = qh * 1024
                        pav = psAV.tile([128, 1024], F32, name=f"pav{h}_{qh}", tag="pav")
                        for kc in range(NKC):
                            ksl = slice(kc * 128, (kc + 1) * 128)
                            ps = psS.tile([128, 1024], F32, name=f"pss{h}_{kc}_{qh}", tag="pss")
                            for qq in range(2):
                                nc.tensor.matmul(
                                    ps[:, qq * 512:(qq + 1) * 512],
                                    kt[ho:ho + 64, ksl],
                                    qt[ho:ho + 64, q0 + qq * 512:q0 + (qq + 1) * 512],
                                    start=True, stop=True,
                                )
                            ex = expp.tile([128, 1024], F32R, name=f"ex{h}_{kc}_{qh}", tag="ex")
                            nc.scalar.activation(ex, ps, EXP, bias=0.0, scale=float(SCALE))
                            if h == 0 and kc == 0 and qh == 0:
                                dbg("dbg_ex0", ex)
                            for qq in range(2):
                                nc.tensor.matmul(
                                    pav[0:65, qq * 512:(qq + 1) * 512],
                                    vg[kc][:, h * 65:(h + 1) * 65],
                                    ex[:, qq * 512:(qq + 1) * 512],
                                    start=(kc == 0), stop=(kc == NKC - 1),
                                )
                        # normalize: aT[head rows] = pav[0:64] / pav[64].
                        # Route through a PE matmul (iden/shift) so results land
                        # on the right partitions AND the PE clock observes DVE,
                        # keeping every fp32r matmul at <=1 sync wait.
                        rcp = rcpp.tile([128, 1024], F32R, name=f"rcp{h}_{qh}", tag="rcp")
                        with nc.allow_low_precision(reason="fp32r has 11-bit mantissa"):
                            nc.vector.reciprocal(rcp[64:65, :], pav[64:65, :])
                        # broadcast 1/den across partitions via a DRAM round
                        # trip (partition_broadcast ignores the source AP's
                        # partition offset on HW)
                        drow = drp.tile([1, 1024], F32R, name=f"dr{h}_{qh}", tag="dr")
                        nc.sync.dma_start(drow, rcp[64:65, :])
                        bc = bcp.tile([128, 1024], F32R, name=f"bc{h}_{qh}", tag="bc")
                        bsrc = bass.AP(tensor=drow.tensor, offset=drow.offset,
                                       ap=[[0, 64]] + [list(p) for p in drow.ap[1:]])
                        nc.sync.dma_start(bc[0:64, :], bsrc)
                        nc.vector.tensor_mul(rcp[0:64, :], pav[0:64, :], bc[0:64, :])
                        if h == 0 and qh == 0:
                            dbg("dbg_bc0", bc[0:64, :])
                            dbg("dbg_rcp0", rcp[0:65, :])
                        mv = shift_t if ho else iden_t
                        for qq in range(2):
                            nc.tensor.matmul(
                                pav[:, qq * 512:(qq + 1) * 512],
                                mv,
                                rcp[0:64, qq * 512:(qq + 1) * 512],
                                start=True, stop=True)
                        nc.vector.tensor_copy(
                            aT[hc][ho:ho + 64, q0:q0 + 1024], pav[ho:ho + 64, :])

            dbg("dbg_aT0", aT[0])

            # ---------------- Phase 3: output projection ----------------
            with contextlib.ExitStack() as p3:
                outp = p3.enter_context(tc.tile_pool(name="outp", bufs=3))
                psP = p3.enter_context(tc.tile_pool(name="psP", bufs=4, space="PSUM"))
                for tcn in range(16):
                    tsl = slice(tcn * 128, (tcn + 1) * 128)
                    osb = outp.tile([128, 1024], F32, name=f"osb{tcn}", tag="osb")
                    for ec in range(2):
                        ps = psP.tile([128, 512], F32, name=f"psp{tcn}_{ec}", tag="psp")
                        for hcc in range(4):
                            nc.tensor.matmul(
                                ps,
                                aT[hcc][:, tsl],
                                wp_t[hcc][:, ec * 512:(ec + 1) * 512],
                                start=(hcc == 0), stop=(hcc == 3),
                            )
                        nc.vector.tensor_add(
                            osb[:, ec * 512:(ec + 1) * 512], ps,
                            bproj_t[:, ec * 512:(ec + 1) * 512])
                    nc.sync.dma_start(out[tsl, :], osb)


def _input_specs():
    # name -> (shape, is_fp32r)
    return {
        "xT": ((C, T), True), "wqkv": ((C, 3 * C // G), True),
        "wproj": ((C // G, C), True),
        "bqk": ((128, 8), False), "bv": ((128, 520), False),
        "bproj": ((128, 1024), False),
        "cosT": ((128, T), False), "sinT": ((128, T), False),
        "rope_perm": ((128, 128), True), "shift64": ((64, 128), True),
        "iden64": ((64, 128), True),
    }


def _build_program():
    import concourse.mybir as mybir
    import concourse.tile as tile
    from concourse import bacc

    nc = bacc.Bacc("TRN2", target_bir_lowering=False, debug=False)
    ins = {}
    for name, (shape, is_r) in _input_specs().items():
        dt = mybir.dt.float32r if is_r else mybir.dt.float32
        ins[name] = nc.dram_tensor(name, list(shape), dt,
                                   kind="ExternalInput").ap()
    outs = {"out": nc.dram_tensor("out", [T, C], mybir.dt.float32,
                                  kind="ExternalOutput").ap()}
    with tile.TileContext(nc) as tc:
        _attn_body(tc, outs, ins)
    nc.compile()
    return nc


def _core_inputs(core, x, W_qkv, b_qkv, W_proj, b_proj, cosT, sinT):
    b, g = divmod(core, 2)
    f32 = np.float32
    xT = _round_fp32r(np.asarray(x[b], dtype=f32).T)
    W_qkv = np.asarray(W_qkv, dtype=f32)
    b_qkv = np.asarray(b_qkv, dtype=f32)
    q = W_qkv[:, g * 512:(g + 1) * 512]
    k = W_qkv[:, C + g * 512:C + (g + 1) * 512]
    v = W_qkv[:, 2 * C + g * 512:2 * C + (g + 1) * 512]
    wqkv = _round_fp32r(np.concatenate([q, k, v], axis=1))
    bq = b_qkv[g * 512:(g + 1) * 512]
    bk = b_qkv[C + g * 512:C + (g + 1) * 512]
    bqk = np.ascontiguousarray(
        np.stack([bq[i * 128:(i + 1) * 128] for i in range(4)]
                 + [bk[i * 128:(i + 1) * 128] for i in range(4)], axis=1))
    bvr = b_qkv[2 * C + g * 512:2 * C + (g + 1) * 512].reshape(8, 64)
    bvg = np.concatenate([bvr, np.ones((8, 1), f32)], axis=1).reshape(-1)  # (8*65,)
    bv = np.ascontiguousarray(np.tile(bvg[None, :], (128, 1)))
    wproj = _round_fp32r(np.asarray(W_proj, dtype=f32)[g * 512:(g + 1) * 512])
    if g == 0:
        bproj = np.ascontiguousarray(
            np.tile(np.asarray(b_proj, dtype=f32)[None, :], (128, 1)))
    else:
        bproj = np.zeros((128, C), dtype=f32)
    P, S, I = _perm_tables()
    return {"xT": xT, "wqkv": wqkv, "wproj": wproj, "bqk": bqk, "bv": bv,
            "bproj": bproj, "cosT": cosT, "sinT": sinT,
            "rope_perm": P, "shift64": S, "iden64": I}


def run(x, W_qkv, b_qkv, W_proj, b_proj, trace=False):
    from concourse.bass_utils import run_bass_kernel_spmd

    if "nc" not in _CACHED:
        _CACHED["nc"] = _build_program()
    nc = _CACHED["nc"]

    cosT, sinT = _rope_tables()
    in_maps = [_core_inputs(c, x, W_qkv, b_qkv, W_proj, b_proj, cosT, sinT)
               for c in range(8)]
    res = run_bass_kernel_spmd(nc, in_maps, core_ids=list(range(8)), trace=trace)
    parts = [r["out"] for r in res.results]
    out = np.stack([parts[2 * b] + parts[2 * b + 1] for b in range(B)], axis=0)
    return out.astype(np.float32), res


def kernel(x, W_qkv, b_qkv, W_proj, b_proj):
    out, _ = run(x, W_qkv, b_qkv, W_proj, b_proj, trace=False)
    return out



# revision 9
# speedup vs baseline: 1.4650x; 1.4650x over previous
"""Multi-head attention (RoPE) Trainium2 Bass kernel — pipelined bf16 version.

Problem: B=4, T=2048, C=1024, H=16, d=64, fp32 in/out, full attention + RoPE.
Sharding: 8 cores = 4 batches x 2 head-groups (8 heads each). Each core
computes its batch's attention for its heads plus the partial (transposed)
output projection; the host sums the two head-group partials per batch and
transposes back.

Design notes (cost-model driven):
- All matmul operands are bf16 (1 cycle/row on PE, half the SBUF/DMA of f32).
- AV uses a token-major dataflow: out[q, 65] = ex_chunk^T @ [ones|v], using
  all 128 output partitions (halves AV PE time vs a 65-partition head-major
  form) and making softmax normalization a per-partition scalar multiply.
  The softmax denominator rides along as column 0 via the ones column of vg.
- Normalized attention output transposes back to head-dim-major via one
  SBUF->SBUF DMA xbar transpose per (head-pair, q-half); PE is not involved.
- One instruction stream software-pipelines everything: QKV chunk
  projections, v-projections and the second-half output projection run as PE
  filler inside the ACT-bound attention stretch so neither PE nor the
  Activation engine (exp) ever starves. RoPE for chunk i-1 is emitted inside
  chunk i's slot so its PE permutation-matmul never waits on DVE.
- Projection is emitted transposed (features on partitions) so its bias is a
  per-partition scalar; the host transposes the final result (untimed).
"""

import numpy as np
import ml_dtypes

B, T, C = 4, 2048, 1024
H, D = 16, 64
G = 2              # head groups (cores per batch)
HG = H // G        # heads per core = 8
CC = C // 128      # 8 contraction chunks
NKC = T // 128     # 16 key chunks
NTB = T // 512     # 4 t-blocks
ROPE_BASE = 10000.0
SCALE = 1.0 / np.sqrt(D)

FUSED_NORM = True      # stride-0 free-dim broadcast of 1/den in one DVE op
TRANSPOSE_3D = True    # one xbar DMA transpose per (pair, q-half)

_CACHED = {}


def _rope_tables():
    inv_freq = 1.0 / (ROPE_BASE ** (np.arange(0, D, 2, dtype=np.float32) / D))
    t = np.arange(T, dtype=np.float32)
    freqs = np.outer(t, inv_freq).astype(np.float32)          # (T, 32)
    emb = np.concatenate([freqs, freqs], axis=-1)             # (T, 64)
    cos = np.cos(emb).T.astype(np.float32)                    # (64, T)
    sin = np.sin(emb).T.astype(np.float32)                    # (64, T)
    cosT = np.concatenate([cos, cos], axis=0)                 # (128, T) two heads/chunk
    sinT = np.concatenate([sin, sin], axis=0)
    return np.ascontiguousarray(cosT), np.ascontiguousarray(sinT)


def _perm_table():
    # rot[d] = sum_s P[s, d] * raw[s] = rotate_half with sign, 2 heads/chunk
    P = np.zeros((128, 128), np.float32)
    for d in range(128):
        blk, dd = divmod(d, D)
        if dd < 32:
            P[blk * D + dd + 32, d] = -1.0
        else:
            P[blk * D + dd - 32, d] = 1.0
    return P


def _attn_body(tc, outs, ins):
    """Tile kernel body. ins/outs are dicts of DRAM APs."""
    import contextlib
    import concourse.bass as bass
    import concourse.mybir as mybir

    nc = tc.nc
    F32 = mybir.dt.float32
    BF16 = mybir.dt.bfloat16
    EXP = mybir.ActivationFunctionType.Exp

    xT = ins["xT"]            # (1024, 2048) bf16  x[b].T
    wqkv = ins["wqkv"]        # (1024, 1536) bf16  [Wq | Wk | Wv] cols for group
    wproj = ins["wproj"]      # (512, 1024) bf16
    bqk = ins["bqk"]          # (128, 8) f32 per-chunk per-partition bias
    bv = ins["bv"]            # (128, 520) f32 broadcast [1|v-bias] per head
    bpr = ins["bpr"]          # (128, 8) f32 proj bias (e-chunk cols; zeros g1)
    cosT_d = ins["cosT"]      # (128, 2048) bf16
    sinT_d = ins["sinT"]      # (128, 2048) bf16
    perm_d = ins["rope_perm"]  # (128, 128) bf16 signed rotate_half permutation
    out = outs["out"]         # (1024, 2048) f32 partial transposed output

    def dbg(name, tile_ap):
        if name in outs:
            nc.sync.dma_start(outs[name].bitcast(tile_ap.dtype), tile_ap)

    ctx = contextlib.ExitStack()
    with ctx:
        pers = ctx.enter_context(tc.tile_pool(name="pers", bufs=1))

        # ---------------- persistent tiles ----------------
        x_t = pers.tile([128, CC * T], BF16, name="x_t", tag="x_t")
        wqk_t = pers.tile([128, CC * 1024], BF16, name="wqk_t", tag="wqk_t")
        wv_t = pers.tile([128, CC * 512], BF16, name="wv_t", tag="wv_t")
        wp_t = pers.tile([128, 4 * 1024], BF16, name="wp_t", tag="wp_t")
        cos_t = pers.tile([128, T], BF16, name="cos_t", tag="cos_t")
        sin_t = pers.tile([128, T], BF16, name="sin_t", tag="sin_t")
        perm_t = pers.tile([128, 128], BF16, name="perm_t", tag="perm_t")
        bqk_t = pers.tile([128, 8], F32, name="bqk_t", tag="bqk_t")
        bv_t = pers.tile([128, 520], F32, name="bv_t", tag="bv_t")
        bpr_t = pers.tile([128, 8], F32, name="bpr_t", tag="bpr_t")
        qk = [pers.tile([128, T], BF16, name=f"qk{j}", tag=f"qk{j}") for j in range(8)]
        vg = [pers.tile([128, HG * 65], BF16, name=f"vg{k}", tag=f"vg{k}") for k in range(NKC)]
        aT = [pers.tile([128, T], BF16, name=f"aT{i}", tag=f"aT{i}") for i in range(4)]

        # ---------------- working pools ----------------
        # PSUM: psS 2x2 banks (scores/exp), psAV 1x2 banks (AV accum),
        # psF 1x2 banks (qkv/v/proj filler groups + rope perm outputs).
        psS = ctx.enter_context(tc.tile_pool(name="psS", bufs=2, space="PSUM"))
        psAV = ctx.enter_context(tc.tile_pool(name="psAV", bufs=1, space="PSUM"))
        psF = ctx.enter_context(tc.tile_pool(name="psF", bufs=1, space="PSUM"))
        expool = ctx.enter_context(tc.tile_pool(name="expool", bufs=4))
        rawp = ctx.enter_context(tc.tile_pool(name="rawp", bufs=2))
        tmpp = ctx.enter_context(tc.tile_pool(name="tmpp", bufs=2))
        tmpcp = ctx.enter_context(tc.tile_pool(name="tmpcp", bufs=2))
        denp = ctx.enter_context(tc.tile_pool(name="denp", bufs=2))
        rcpp = ctx.enter_context(tc.tile_pool(name="rcpp", bufs=2))
        avnp = ctx.enter_context(tc.tile_pool(name="avnp", bufs=2))
        osbp = ctx.enter_context(tc.tile_pool(name="osbp", bufs=8))

        # ---------------- input DMAs (no waits; ordered for earliest use) ---
        x3d = x_t.rearrange("p (c t) -> p c t", t=T)
        xTd = xT.rearrange("(c p) t -> p c t", p=128)
        wqk3 = wqk_t.rearrange("p (c e) -> p c e", e=1024)
        wq_d = wqkv[:, 0:512].rearrange("(c p) e -> p c e", p=128)
        wk_d = wqkv[:, 512:1024].rearrange("(c p) e -> p c e", p=128)
        nc.sync.dma_start(bqk_t, bqk)
        nc.sync.dma_start(wqk3[:, :, 0:512], wq_d)
        nc.sync.dma_start(x3d[:, :, 0:512], xTd[:, :, 0:512])
        nc.sync.dma_start(cos_t, cosT_d)
        nc.sync.dma_start(sin_t, sinT_d)
        nc.sync.dma_start(perm_t, perm_d)
        nc.sync.dma_start(x3d[:, :, 512:1024], xTd[:, :, 512:1024])
        nc.sync.dma_start(wqk3[:, :, 512:1024], wk_d)
        nc.sync.dma_start(x3d[:, :, 1024:1536], xTd[:, :, 1024:1536])
        nc.sync.dma_start(x3d[:, :, 1536:2048], xTd[:, :, 1536:2048])
        nc.sync.dma_start(wv_t.rearrange("p (c e) -> p c e", e=512),
                          wqkv[:, 1024:1536].rearrange("(c p) e -> p c e", p=128))
        nc.sync.dma_start(bv_t, bv)
        nc.sync.dma_start(bpr_t, bpr)
        nc.sync.dma_start(wp_t.rearrange("p (i e) -> p i e", e=1024),
                          wproj.rearrange("(i p) e -> p i e", p=128))

        # ---------------- emitters ----------------
        pend = {"rope": None}  # (jc, tb, raw) awaiting perm-matmul + combine

        def emit_rope(ps_half):
            """Emit pending RoPE combine: perm-matmul into ps_half (psum
            (128,512) f32 slice), then DVE combine into qk[jc]."""
            jc, tb, raw = pend["rope"]
            pend["rope"] = None
            tsl = slice(tb * 512, (tb + 1) * 512)
            nc.tensor.matmul(ps_half, perm_t, raw, start=True, stop=True)
            tmp = tmpp.tile([128, 512], F32, name=f"tm{jc}_{tb}", tag="tmp")
            nc.vector.tensor_mul(tmp, ps_half, sin_t[:, tsl])
            tmpc = tmpcp.tile([128, 512], F32, name=f"tc{jc}_{tb}", tag="tmpc")
            nc.vector.tensor_mul(tmpc, raw, cos_t[:, tsl])
            nc.vector.tensor_add(qk[jc][:, tsl], tmp, tmpc)

        def emit_qk_slot(jc, tb, pool):
            """8 projection matmuls for q/k chunk jc, t-block tb, plus the
            RoPE combine of the previously emitted chunk."""
            col0 = (jc % 4) * 128 + (512 if jc >= 4 else 0)
            tsl = slice(tb * 512, (tb + 1) * 512)
            ps = pool.tile([128, 1024], F32, name=f"psq{jc}_{tb}", tag="s")
            if pend["rope"] is not None:
                emit_rope(ps[:, 512:1024])
            for c in range(CC):
                nc.tensor.matmul(
                    ps[:, 0:512], wqk_t[:, c * 1024 + col0:c * 1024 + col0 + 128],
                    x_t[:, c * T + tb * 512:c * T + (tb + 1) * 512],
                    start=(c == 0), stop=(c == CC - 1))
            raw = rawp.tile([128, 512], BF16, name=f"raw{jc}_{tb}", tag="raw")
            nc.vector.tensor_scalar_add(raw, ps[:, 0:512], bqk_t[:, jc:jc + 1])
            pend["rope"] = (jc, tb, raw)

        def emit_rope_flush(pool):
            ps = pool.tile([128, 1024], F32, name="psflush", tag="s")
            emit_rope(ps[:, 512:1024])

        def emit_v(kc, pool):
            """v for token chunk kc -> vg[kc] = [1|v] per head, bf16."""
            ps = pool.tile([128, 1024], F32, name=f"psv{kc}", tag="s")
            for c in range(CC):
                nc.tensor.matmul(
                    ps[:, 0:512], x_t[:, c * T + kc * 128:c * T + (kc + 1) * 128],
                    wv_t[:, c * 512:(c + 1) * 512],
                    start=(c == 0), stop=(c == CC - 1))
            vv = vg[kc].rearrange("p (g w) -> p g w", w=65)
            bvv = bv_t.rearrange("p (g w) -> p g w", w=65)
            psg = ps[:, 0:512].rearrange("p (g d) -> p g d", d=64)
            nc.vector.tensor_add(vv[:, :, 1:65], psg, bvv[:, :, 1:65])
            nc.vector.tensor_copy(vv[:, :, 0:1], bvv[:, :, 0:1])

        def emit_proj(ec, th, pool):
            """transposed proj: out rows = e-chunk ec, cols = tok block th."""
            tsl = slice(th * 512, (th + 1) * 512)
            ps = pool.tile([128, 1024], F32, name=f"psp{ec}_{th}", tag="s")
            for i in range(4):
                nc.tensor.matmul(
                    ps[:, 0:512], wp_t[:, i * 1024 + ec * 128:i * 1024 + (ec + 1) * 128],
                    aT[i][:, tsl], start=(i == 0), stop=(i == 3))
            osb = osbp.tile([128, 512], F32, name=f"osb{ec}_{th}", tag="osb")
            nc.vector.tensor_scalar_add(osb, ps[:, 0:512], bpr_t[:, ec:ec + 1])
            nc.sync.dma_start(out[ec * 128:(ec + 1) * 128, tsl], osb)

        def emit_av(pav, ex, kc, h):
            # start=True zeroes the whole PSUM bank, so only the first group
            # of each bank (qc 0 and 4) may set it; the bank-wide zero covers
            # the other interleaved accumulation groups' regions.
            mv = vg[kc][:, h * 65:(h + 1) * 65]
            for qc in range(8):
                nc.tensor.matmul(
                    pav[:, qc * 128:qc * 128 + 65],
                    ex[:, qc * 128:(qc + 1) * 128], mv,
                    start=(kc == 0 and qc % 4 == 0), stop=(kc == NKC - 1))

        # filler items per unit index (u = 4*hc + 2*qh + p); each item is a
        # callable taking the psum pool to use.
        def qk_item(jc, tb):
            return lambda: emit_qk_slot(jc, tb, psF)

        flush = lambda: emit_rope_flush(psF)
        unit_fill = {u: [] for u in range(16)}
        unit_fill[0] = [flush]
        unit_fill[1] = [qk_item(1, 0), qk_item(1, 1), qk_item(1, 2)]
        unit_fill[2] = [qk_item(1, 3), qk_item(5, 0), qk_item(5, 1)]
        unit_fill[3] = [qk_item(5, 2), qk_item(5, 3)]
        unit_fill[4] = [flush, qk_item(2, 0), qk_item(2, 1)]
        unit_fill[5] = [qk_item(2, 2), qk_item(2, 3)]
        unit_fill[6] = [qk_item(6, 0), qk_item(6, 1)]
        unit_fill[7] = [qk_item(6, 2), qk_item(6, 3)]
        unit_fill[8] = [flush, qk_item(3, 0), qk_item(3, 1)]
        unit_fill[9] = [qk_item(3, 2), qk_item(3, 3)]
        unit_fill[10] = [qk_item(7, 0), qk_item(7, 1)]
        unit_fill[11] = [qk_item(7, 2), qk_item(7, 3)]
        unit_fill[12] = [flush]
        unit_fill[14] = [(lambda ec: (lambda: emit_proj(ec, 0, psF)))(ec) for ec in range(8)]
        unit_fill[15] = [(lambda ec: (lambda: emit_proj(ec, 1, psF)))(ec) for ec in range(8)]

        # ---------------- preamble PE work (tb-major; V under DMA waits) ---
        pre_pools = [psS, psS, psF]
        pi = 0

        def pre_slot(jc, tb):
            nonlocal pi
            emit_qk_slot(jc, tb, pre_pools[pi % 3])
            pi += 1

        pre_slot(0, 0)
        pre_slot(4, 0)
        pre_slot(0, 1)
        pre_slot(4, 1)
        emit_v(0, psF)
        emit_v(1, psF)
        pre_slot(0, 2)
        pre_slot(4, 2)
        emit_v(2, psF)
        emit_v(3, psF)
        pre_slot(0, 3)
        pre_slot(4, 3)
        emit_v(4, psF)
        emit_v(5, psF)

        # ---------------- attention units ----------------
        for hc in range(4):
            for qh in range(2):
                avn = avnp.tile([128, 1024], BF16, name=f"avn{hc}_{qh}", tag="avn")
                avn3 = avn.rearrange("p (tc w) -> p tc w", w=128)
                for p in range(2):
                    h = 2 * hc + p
                    u = 4 * hc + 2 * qh + p
                    ho = p * 64
                    qt = qk[hc]
                    kt = qk[4 + hc]
                    fills = unit_fill[u]
                    nfill = len(fills)
                    step = max(1, NKC // nfill) if nfill else NKC + 1
                    pav = psAV.tile([128, 1024], F32, name=f"pav{h}_{qh}", tag="pav")
                    exs = [None] * NKC
                    fi = 0
                    for kc in range(NKC):
                        if u == 0 and 6 <= kc <= 15:
                            emit_v(kc, psF)
                        if fi < nfill and kc % step == 0:
                            fills[fi]()
                            fi += 1
                        s = psS.tile([128, 1024], F32, name=f"s{h}_{qh}_{kc}", tag="s")
                        ksl = slice(kc * 128, (kc + 1) * 128)
                        for qq in range(2):
                            qsl = slice(qh * 1024 + qq * 512, qh * 1024 + (qq + 1) * 512)
                            nc.tensor.matmul(
                                s[:, qq * 512:(qq + 1) * 512],
                                kt[ho:ho + 64, ksl], qt[ho:ho + 64, qsl],
                                start=True, stop=True)
                        ex = expool.tile([128, 1024], BF16, name=f"ex{h}_{qh}_{kc}", tag="ex")
                        nc.scalar.activation(ex, s, EXP, bias=0.0, scale=float(SCALE))
                        exs[kc] = ex
                        if kc > 0:
                            emit_av(pav, exs[kc - 1], kc - 1, h)
                    while fi < nfill:
                        fills[fi]()
                        fi += 1
                    emit_av(pav, exs[NKC - 1], NKC - 1, h)
                    if u == 0:
                        dbg("dbg_ex0", exs[0])
                        dbg("dbg_pav0", pav)

                    # normalize: avn[:, tc, ho+d] = pav[:, tc, 1+d] / pav[:, tc, 0]
                    pavr = pav.rearrange("p (qc w) -> p qc w", w=128)
                    den = denp.tile([128, 8], F32, name=f"den{h}_{qh}", tag="den")
                    den3 = den.rearrange("p (a b) -> p a b", b=1)
                    nc.vector.tensor_copy(den3, pavr[:, :, 0:1])
                    rcp = rcpp.tile([128, 8], F32, name=f"rcp{h}_{qh}", tag="rcp")
                    nc.vector.reciprocal(rcp, den)
                    if FUSED_NORM:
                        rcp_b = bass.AP(tensor=rcp.tensor, offset=rcp.offset,
                                        ap=[list(rcp.ap[0]), [1, 8], [0, 64]])
                        nc.vector.tensor_mul(avn3[:, :, ho:ho + 64], pavr[:, :, 1:65], rcp_b)
                    else:
                        for qc in range(8):
                            nc.vector.tensor_scalar_mul(
                                avn3[:, qc:qc + 1, ho:ho + 64],
                                pavr[:, qc:qc + 1, 1:65], rcp[:, qc:qc + 1])
                    if u == 0:
                        dbg("dbg_avn0", avn)
                # pair complete: transpose back to head-dim-major layout
                aT3 = aT[hc].rearrange("p (tc t) -> p tc t", t=128)
                if TRANSPOSE_3D:
                    nc.sync.dma_start_transpose(
                        aT3[:, qh * 8:(qh + 1) * 8, :], avn)
                else:
                    for tcn in range(8):
                        nc.sync.dma_start_transpose(
                            aT[hc][:, qh * 1024 + tcn * 128:qh * 1024 + (tcn + 1) * 128],
                            avn[:, tcn * 128:(tcn + 1) * 128])
        dbg("dbg_aT0", aT[0])

        # ---------------- tail: second half of projection ----------------
        tail_pools = [psS, psS, psF]
        for i, (th, ec) in enumerate([(th, ec) for th in (2, 3) for ec in range(8)]):
            emit_proj(ec, th, tail_pools[i % 3])


def _input_specs():
    # name -> (shape, dtype_str)
    return {
        "xT": ((C, T), "bf16"), "wqkv": ((C, 3 * C // G), "bf16"),
        "wproj": ((C // G, C), "bf16"),
        "bqk": ((128, 8), "f32"), "bv": ((128, 520), "f32"),
        "bpr": ((128, 8), "f32"),
        "cosT": ((128, T), "bf16"), "sinT": ((128, T), "bf16"),
        "rope_perm": ((128, 128), "bf16"),
    }


def _build_program():
    import concourse.mybir as mybir
    import concourse.tile as tile
    from concourse import bacc

    nc = bacc.Bacc("TRN2", target_bir_lowering=False, debug=False)
    ins = {}
    for name, (shape, dts) in _input_specs().items():
        dt = mybir.dt.bfloat16 if dts == "bf16" else mybir.dt.float32
        ins[name] = nc.dram_tensor(name, list(shape), dt,
                                   kind="ExternalInput").ap()
    outs = {"out": nc.dram_tensor("out", [C, T], mybir.dt.float32,
                                  kind="ExternalOutput").ap()}
    with tile.TileContext(nc) as tc:
        _attn_body(tc, outs, ins)
    nc.compile()
    return nc


def _core_inputs(core, x, W_qkv, b_qkv, W_proj, b_proj, cosT, sinT, P):
    b, g = divmod(core, 2)
    f32 = np.float32
    bf16 = ml_dtypes.bfloat16
    xT = np.ascontiguousarray(np.asarray(x[b], dtype=f32).T).astype(bf16)
    W_qkv = np.asarray(W_qkv, dtype=f32)
    b_qkv = np.asarray(b_qkv, dtype=f32)
    q = W_qkv[:, g * 512:(g + 1) * 512]
    k = W_qkv[:, C + g * 512:C + (g + 1) * 512]
    v = W_qkv[:, 2 * C + g * 512:2 * C + (g + 1) * 512]
    wqkv = np.ascontiguousarray(np.concatenate([q, k, v], axis=1)).astype(bf16)
    bq = b_qkv[g * 512:(g + 1) * 512]
    bk = b_qkv[C + g * 512:C + (g + 1) * 512]
    bqk = np.ascontiguousarray(
        np.stack([bq[i * 128:(i + 1) * 128] for i in range(4)]
                 + [bk[i * 128:(i + 1) * 128] for i in range(4)], axis=1))
    bvr = b_qkv[2 * C + g * 512:2 * C + (g + 1) * 512].reshape(HG, 64)
    bvg = np.concatenate([np.ones((HG, 1), f32), bvr], axis=1).reshape(-1)  # (520,)
    bv = np.ascontiguousarray(np.tile(bvg[None, :], (128, 1)))
    wproj = np.ascontiguousarray(
        np.asarray(W_proj, dtype=f32)[g * 512:(g + 1) * 512]).astype(bf16)
    if g == 0:
        bpr = np.ascontiguousarray(
            np.asarray(b_proj, dtype=f32).reshape(8, 128).T)
    else:
        bpr = np.zeros((128, 8), dtype=f32)
    return {"xT": xT, "wqkv": wqkv, "wproj": wproj, "bqk": bqk, "bv": bv,
            "bpr": bpr, "cosT": cosT, "sinT": sinT, "rope_perm": P}


def run(x, W_qkv, b_qkv, W_proj, b_proj, trace=False):
    from concourse.bass_utils import run_bass_kernel_spmd

    if "nc" not in _CACHED:
        _CACHED["nc"] = _build_program()
    nc = _CACHED["nc"]

    bf16 = ml_dtypes.bfloat16
    cosT, sinT = _rope_tables()
    cosT = cosT.astype(bf16)
    sinT = sinT.astype(bf16)
    P = _perm_table().astype(bf16)
    in_maps = [_core_inputs(c, x, W_qkv, b_qkv, W_proj, b_proj, cosT, sinT, P)
               for c in range(8)]
    res = run_bass_kernel_spmd(nc, in_maps, core_ids=list(range(8)), trace=trace)
    parts = [np.asarray(r["out"], dtype=np.float32) for r in res.results]
    out = np.stack([(parts[2 * b] + parts[2 * b + 1]).T for b in range(B)], axis=0)
    return np.ascontiguousarray(out), res


def kernel(x, W_qkv, b_qkv, W_proj, b_proj):
    out, _ = run(x, W_qkv, b_qkv, W_proj, b_proj, trace=False)
    return out


# revision 17
# speedup vs baseline: 1.5363x; 1.0486x over previous
"""Multi-head attention (RoPE) Trainium2 Bass kernel — pipelined bf16 version.

Problem: B=4, T=2048, C=1024, H=16, d=64, fp32 in/out, full attention + RoPE.
Sharding: 8 cores = 4 batches x 2 head-groups (8 heads each). Each core
computes its batch's attention for its heads plus the partial (transposed)
output projection; the host sums the two head-group partials per batch and
transposes back.

Design notes (cost-model driven):
- All matmul operands are bf16 (1 cycle/row on PE, half the SBUF/DMA of f32).
- AV uses a token-major dataflow: out[q, 65] = ex_chunk^T @ [ones|v], using
  all 128 output partitions (halves AV PE time vs a 65-partition head-major
  form) and making softmax normalization a per-partition scalar multiply.
  The softmax denominator rides along as column 0 via the ones column of vg.
- Normalized attention output transposes back to head-dim-major via one
  SBUF->SBUF DMA xbar transpose per (head-pair, q-half); PE is not involved.
- One instruction stream software-pipelines everything: QKV chunk
  projections, v-projections and the second-half output projection run as PE
  filler inside the ACT-bound attention stretch so neither PE nor the
  Activation engine (exp) ever starves. RoPE for chunk i-1 is emitted inside
  chunk i's slot so its PE permutation-matmul never waits on DVE.
- Projection is emitted transposed (features on partitions) so its bias is a
  per-partition scalar; the host transposes the final result (untimed).
"""

import numpy as np
import ml_dtypes

B, T, C = 4, 2048, 1024
H, D = 16, 64
G = 2              # head groups (cores per batch)
HG = H // G        # heads per core = 8
CC = C // 128      # 8 contraction chunks
NKC = T // 128     # 16 key chunks
NTB = T // 512     # 4 t-blocks
ROPE_BASE = 10000.0
SCALE = 1.0 / np.sqrt(D)

FUSED_NORM = True      # stride-0 free-dim broadcast of 1/den in one DVE op
TRANSPOSE_3D = True    # one xbar DMA transpose per (pair, q-half)

_CACHED = {}


def _rope_tables():
    inv_freq = 1.0 / (ROPE_BASE ** (np.arange(0, D, 2, dtype=np.float32) / D))
    t = np.arange(T, dtype=np.float32)
    freqs = np.outer(t, inv_freq).astype(np.float32)          # (T, 32)
    emb = np.concatenate([freqs, freqs], axis=-1)             # (T, 64)
    cos = np.cos(emb).T.astype(np.float32)                    # (64, T)
    sin = np.sin(emb).T.astype(np.float32)                    # (64, T)
    cosT = np.concatenate([cos, cos], axis=0)                 # (128, T) two heads/chunk
    sinT = np.concatenate([sin, sin], axis=0)
    return np.ascontiguousarray(cosT), np.ascontiguousarray(sinT)


def _perm_table():
    # rot[d] = sum_s P[s, d] * raw[s] = rotate_half with sign, 2 heads/chunk
    P = np.zeros((128, 128), np.float32)
    for d in range(128):
        blk, dd = divmod(d, D)
        if dd < 32:
            P[blk * D + dd + 32, d] = -1.0
        else:
            P[blk * D + dd - 32, d] = 1.0
    return P


def _attn_body(tc, outs, ins):
    """Tile kernel body. ins/outs are dicts of DRAM APs."""
    import contextlib
    import concourse.bass as bass
    import concourse.mybir as mybir

    nc = tc.nc
    F32 = mybir.dt.float32
    BF16 = mybir.dt.bfloat16
    EXP = mybir.ActivationFunctionType.Exp

    xT = ins["xT"]            # (1024, 2048) bf16  x[b].T
    wqkv = ins["wqkv"]        # (1024, 1536) bf16  [Wq | Wk | Wv] cols for group
    wproj = ins["wproj"]      # (512, 1024) bf16
    bqk = ins["bqk"]          # (128, 8) f32 per-chunk per-partition bias
    bv = ins["bv"]            # (128, 520) f32 broadcast [1|v-bias] per head
    bpr = ins["bpr"]          # (128, 8) f32 proj bias (e-chunk cols; zeros g1)
    cosT_d = ins["cosT"]      # (128, 2048) bf16
    sinT_d = ins["sinT"]      # (128, 2048) bf16
    perm_d = ins["rope_perm"]  # (128, 128) bf16 signed rotate_half permutation
    out = outs["out"]         # (1024, 2048) f32 partial transposed output

    def dbg(name, tile_ap):
        if name in outs:
            nc.sync.dma_start(outs[name].bitcast(tile_ap.dtype), tile_ap)

    ctx = contextlib.ExitStack()
    with ctx:
        pers = ctx.enter_context(tc.tile_pool(name="pers", bufs=1))

        # ---------------- persistent tiles ----------------
        x_t = pers.tile([128, CC * T], BF16, name="x_t", tag="x_t")
        wqk_t = pers.tile([128, CC * 1024], BF16, name="wqk_t", tag="wqk_t")
        wv_t = pers.tile([128, CC * 512], BF16, name="wv_t", tag="wv_t")
        wp_t = pers.tile([128, 4 * 1024], BF16, name="wp_t", tag="wp_t")
        cos_t = pers.tile([128, T], BF16, name="cos_t", tag="cos_t")
        sin_t = pers.tile([128, T], BF16, name="sin_t", tag="sin_t")
        perm_t = pers.tile([128, 128], BF16, name="perm_t", tag="perm_t")
        bqk_t = pers.tile([128, 8], F32, name="bqk_t", tag="bqk_t")
        bv_t = pers.tile([128, 520], F32, name="bv_t", tag="bv_t")
        bpr_t = pers.tile([128, 8], F32, name="bpr_t", tag="bpr_t")
        qk = [pers.tile([128, T], BF16, name=f"qk{j}", tag=f"qk{j}") for j in range(8)]
        vg = [pers.tile([128, HG * 65], BF16, name=f"vg{k}", tag=f"vg{k}") for k in range(NKC)]
        aT = [pers.tile([128, T], BF16, name=f"aT{i}", tag=f"aT{i}") for i in range(4)]

        # ---------------- working pools ----------------
        # PSUM: psS 2x2 banks (scores/exp), psAV 1x2 banks (AV accum),
        # psF 1x2 banks (qkv/v/proj filler groups + rope perm outputs).
        psS = ctx.enter_context(tc.tile_pool(name="psS", bufs=2, space="PSUM"))
        psAV = ctx.enter_context(tc.tile_pool(name="psAV", bufs=1, space="PSUM"))
        psF = ctx.enter_context(tc.tile_pool(name="psF", bufs=1, space="PSUM"))
        expool = ctx.enter_context(tc.tile_pool(name="expool", bufs=4))
        rawp = ctx.enter_context(tc.tile_pool(name="rawp", bufs=2))
        tmpp = ctx.enter_context(tc.tile_pool(name="tmpp", bufs=2))
        tmpcp = ctx.enter_context(tc.tile_pool(name="tmpcp", bufs=2))
        denp = ctx.enter_context(tc.tile_pool(name="denp", bufs=2))
        rcpp = ctx.enter_context(tc.tile_pool(name="rcpp", bufs=2))
        avnp = ctx.enter_context(tc.tile_pool(name="avnp", bufs=2))
        osbp = ctx.enter_context(tc.tile_pool(name="osbp", bufs=8))

        # ---------------- input DMAs (no waits; ordered for earliest use) ---
        x3d = x_t.rearrange("p (c t) -> p c t", t=T)
        xTd = xT.rearrange("(c p) t -> p c t", p=128)
        wqk3 = wqk_t.rearrange("p (c e) -> p c e", e=1024)

        def w_slice(jc):
            # per-chunk 128-col slice of [Wq|Wk] for q/k chunk jc
            col0 = (jc % 4) * 128 + (512 if jc >= 4 else 0)
            nc.sync.dma_start(
                wqk3[:, :, col0:col0 + 128],
                wqkv[:, col0:col0 + 128].rearrange("(c p) e -> p c e", p=128))

        nc.sync.dma_start(perm_t, perm_d)
        w_slice(0)
        nc.sync.dma_start(x3d[:, :, 0:512], xTd[:, :, 0:512])
        nc.sync.dma_start(bqk_t, bqk)
        nc.sync.dma_start(cos_t, cosT_d)
        nc.sync.dma_start(sin_t, sinT_d)
        w_slice(4)
        nc.sync.dma_start(x3d[:, :, 512:1024], xTd[:, :, 512:1024])
        nc.sync.dma_start(wv_t.rearrange("p (c e) -> p c e", e=512),
                          wqkv[:, 1024:1536].rearrange("(c p) e -> p c e", p=128))
        nc.sync.dma_start(bv_t, bv)
        nc.sync.dma_start(x3d[:, :, 1024:1536], xTd[:, :, 1024:1536])
        nc.sync.dma_start(x3d[:, :, 1536:2048], xTd[:, :, 1536:2048])
        for jc in (1, 5, 2, 6, 3, 7):
            w_slice(jc)
        nc.sync.dma_start(bpr_t, bpr)
        nc.sync.dma_start(wp_t.rearrange("p (i e) -> p i e", e=1024),
                          wproj.rearrange("(i p) e -> p i e", p=128))

        # PE clock warmup: keep the tensor engine continuously busy from the
        # moment perm_t lands until the first real slot's inputs arrive, so
        # the p-state model reaches full clock before real work dispatches.
        warm = psF.tile([128, 1024], F32, name="warm", tag="s")
        for i in range(40):
            nc.tensor.matmul(warm[:, 0:128], perm_t, perm_t, start=True, stop=True)

        # ---------------- emitters ----------------
        pend = {"rope": None}  # (jc, tb, raw) awaiting perm-matmul + combine

        def emit_rope(ps_half):
            """Emit pending RoPE combine: perm-matmul into ps_half (psum
            (128,512) f32 slice), then DVE combine into qk[jc]."""
            jc, tb, raw = pend["rope"]
            pend["rope"] = None
            tsl = slice(tb * 512, (tb + 1) * 512)
            nc.tensor.matmul(ps_half, perm_t, raw, start=True, stop=True)
            tmp = tmpp.tile([128, 512], F32, name=f"tm{jc}_{tb}", tag="tmp")
            nc.vector.tensor_mul(tmp, ps_half, sin_t[:, tsl])
            tmpc = tmpcp.tile([128, 512], F32, name=f"tc{jc}_{tb}", tag="tmpc")
            nc.vector.tensor_mul(tmpc, raw, cos_t[:, tsl])
            nc.vector.tensor_add(qk[jc][:, tsl], tmp, tmpc)

        def emit_qk_slot(jc, tb, pool):
            """8 projection matmuls for q/k chunk jc, t-block tb, plus the
            RoPE combine of the previously emitted chunk."""
            col0 = (jc % 4) * 128 + (512 if jc >= 4 else 0)
            tsl = slice(tb * 512, (tb + 1) * 512)
            ps = pool.tile([128, 1024], F32, name=f"psq{jc}_{tb}", tag="s")
            if pend["rope"] is not None:
                emit_rope(ps[:, 512:1024])
            for c in range(CC):
                nc.tensor.matmul(
                    ps[:, 0:512], wqk_t[:, c * 1024 + col0:c * 1024 + col0 + 128],
                    x_t[:, c * T + tb * 512:c * T + (tb + 1) * 512],
                    start=(c == 0), stop=(c == CC - 1))
            raw = rawp.tile([128, 512], BF16, name=f"raw{jc}_{tb}", tag="raw")
            nc.vector.tensor_scalar_add(raw, ps[:, 0:512], bqk_t[:, jc:jc + 1])
            pend["rope"] = (jc, tb, raw)

        def emit_rope_flush(pool):
            ps = pool.tile([128, 1024], F32, name="psflush", tag="s")
            emit_rope(ps[:, 512:1024])

        def emit_v(kc, pool):
            """v for token chunk kc -> vg[kc] = [1|v] per head, bf16."""
            ps = pool.tile([128, 1024], F32, name=f"psv{kc}", tag="s")
            for c in range(CC):
                nc.tensor.matmul(
                    ps[:, 0:512], x_t[:, c * T + kc * 128:c * T + (kc + 1) * 128],
                    wv_t[:, c * 512:(c + 1) * 512],
                    start=(c == 0), stop=(c == CC - 1))
            vv = vg[kc].rearrange("p (g w) -> p g w", w=65)
            bvv = bv_t.rearrange("p (g w) -> p g w", w=65)
            psg = ps[:, 0:512].rearrange("p (g d) -> p g d", d=64)
            nc.vector.tensor_add(vv[:, :, 1:65], psg, bvv[:, :, 1:65])
            nc.vector.tensor_copy(vv[:, :, 0:1], bvv[:, :, 0:1])

        def emit_proj(ec, th, pool):
            """transposed proj: out rows = e-chunk ec, cols = tok block th."""
            tsl = slice(th * 512, (th + 1) * 512)
            ps = pool.tile([128, 1024], F32, name=f"psp{ec}_{th}", tag="s")
            for i in range(4):
                nc.tensor.matmul(
                    ps[:, 0:512], wp_t[:, i * 1024 + ec * 128:i * 1024 + (ec + 1) * 128],
                    aT[i][:, tsl], start=(i == 0), stop=(i == 3))
            osb = osbp.tile([128, 512], F32, name=f"osb{ec}_{th}", tag="osb")
            nc.vector.tensor_scalar_add(osb, ps[:, 0:512], bpr_t[:, ec:ec + 1])
            nc.sync.dma_start(out[ec * 128:(ec + 1) * 128, tsl], osb)

        def emit_av(pav, ex, kc, h):
            # start=True zeroes the whole PSUM bank, so only the first group
            # of each bank (qc 0 and 4) may set it; the bank-wide zero covers
            # the other interleaved accumulation groups' regions.
            mv = vg[kc][:, h * 65:(h + 1) * 65]
            for qc in range(8):
                nc.tensor.matmul(
                    pav[:, qc * 128:qc * 128 + 65],
                    ex[:, qc * 128:(qc + 1) * 128], mv,
                    start=(kc == 0 and qc % 4 == 0), stop=(kc == NKC - 1))

        # filler items per unit index (u = 4*hc + 2*qh + p); each item is a
        # callable taking the psum pool to use.
        def qk_item(jc, tb):
            return lambda: emit_qk_slot(jc, tb, psF)

        flush = lambda: emit_rope_flush(psF)
        unit_fill = {u: [] for u in range(16)}
        unit_fill[1] = [qk_item(1, 0), qk_item(1, 1), qk_item(1, 2)]
        unit_fill[2] = [qk_item(1, 3), qk_item(5, 0), qk_item(5, 1)]
        unit_fill[3] = [qk_item(5, 2), qk_item(5, 3)]
        unit_fill[4] = [flush, qk_item(2, 0), qk_item(2, 1)]
        unit_fill[5] = [qk_item(2, 2), qk_item(2, 3)]
        unit_fill[6] = [qk_item(6, 0), qk_item(6, 1)]
        unit_fill[7] = [qk_item(6, 2), qk_item(6, 3)]
        unit_fill[8] = [flush, qk_item(3, 0), qk_item(3, 1)]
        unit_fill[9] = [qk_item(3, 2), qk_item(3, 3)]
        unit_fill[10] = [qk_item(7, 0), qk_item(7, 1)]
        unit_fill[11] = [qk_item(7, 2), qk_item(7, 3)]
        unit_fill[12] = [flush]
        unit_fill[14] = [(lambda ec: (lambda: emit_proj(ec, 0, psF)))(ec) for ec in range(8)]
        unit_fill[15] = [(lambda ec: (lambda: emit_proj(ec, 1, psF)))(ec) for ec in range(8)]

        # ---------------- minimal preamble: just enough for S(h0,qh0,kc=0) ---
        # qk[0] tb0+tb1 and qk[4] tb0, fully roped (rope of chunk i rides in
        # slot i+1; one flush closes the chain).
        emit_qk_slot(0, 0, psS)
        emit_qk_slot(4, 0, psS)
        emit_qk_slot(0, 1, psS)
        emit_rope_flush(psS)

        # unit 0 in-loop schedule: remaining qk slots + v chunks as filler;
        # AV lags exp by 3 so early A's don't stall on the wv/x DMAs.
        u0_slots = {0: [lambda: emit_qk_slot(4, 1, psS)],
                    1: [lambda: emit_qk_slot(0, 2, psS)],
                    2: [lambda: emit_qk_slot(4, 2, psS), lambda: emit_v(0, psF)],
                    3: [lambda: emit_qk_slot(0, 3, psS), lambda: emit_v(1, psF)],
                    4: [lambda: emit_qk_slot(4, 3, psS), lambda: emit_v(2, psF)],
                    5: [lambda: emit_rope_flush(psS), lambda: emit_v(3, psF)]}
        for kc in range(6, NKC):
            u0_slots[kc] = [(lambda k: (lambda: emit_v(k, psF)))(kc - 2)]

        # ---------------- attention units ----------------
        for hc in range(4):
            for qh in range(2):
                avn = avnp.tile([128, 1024], BF16, name=f"avn{hc}_{qh}", tag="avn")
                avn3 = avn.rearrange("p (tc w) -> p tc w", w=128)
                for p in range(2):
                    h = 2 * hc + p
                    u = 4 * hc + 2 * qh + p
                    ho = p * 64
                    qt = qk[hc]
                    kt = qk[4 + hc]
                    fills = unit_fill[u]
                    nfill = len(fills)
                    step = max(1, NKC // nfill) if nfill else NKC + 1
                    av_lag = 3 if u == 0 else 1
                    pav = psAV.tile([128, 1024], F32, name=f"pav{h}_{qh}", tag="pav")
                    exs = [None] * NKC
                    fi = 0
                    for kc in range(NKC):
                        s = psS.tile([128, 1024], F32, name=f"s{h}_{qh}_{kc}", tag="s")
                        ksl = slice(kc * 128, (kc + 1) * 128)
                        for qq in range(2):
                            qsl = slice(qh * 1024 + qq * 512, qh * 1024 + (qq + 1) * 512)
                            nc.tensor.matmul(
                                s[:, qq * 512:(qq + 1) * 512],
                                kt[ho:ho + 64, ksl], qt[ho:ho + 64, qsl],
                                start=True, stop=True)
                        ex = expool.tile([128, 1024], BF16, name=f"ex{h}_{qh}_{kc}", tag="ex")
                        nc.scalar.activation(ex, s, EXP, bias=0.0, scale=float(SCALE))
                        exs[kc] = ex
                        if u == 0:
                            for item in u0_slots.get(kc, ()):
                                item()
                        elif fi < nfill and kc % step == 0:
                            fills[fi]()
                            fi += 1
                        if kc >= av_lag:
                            emit_av(pav, exs[kc - av_lag], kc - av_lag, h)
                    while fi < nfill:
                        fills[fi]()
                        fi += 1
                    if u == 0:
                        emit_v(14, psF)
                        emit_v(15, psF)
                    for kc in range(NKC - av_lag, NKC):
                        emit_av(pav, exs[kc], kc, h)
                    if u == 0:
                        dbg("dbg_ex0", exs[0])
                        dbg("dbg_pav0", pav)

                    # normalize: avn[:, tc, ho+d] = pav[:, tc, 1+d] / pav[:, tc, 0]
                    pavr = pav.rearrange("p (qc w) -> p qc w", w=128)
                    den = denp.tile([128, 8], F32, name=f"den{h}_{qh}", tag="den")
                    den3 = den.rearrange("p (a b) -> p a b", b=1)
                    nc.vector.tensor_copy(den3, pavr[:, :, 0:1])
                    rcp = rcpp.tile([128, 8], F32, name=f"rcp{h}_{qh}", tag="rcp")
                    nc.vector.reciprocal(rcp, den)
                    if FUSED_NORM:
                        rcp_b = bass.AP(tensor=rcp.tensor, offset=rcp.offset,
                                        ap=[list(rcp.ap[0]), [1, 8], [0, 64]])
                        nc.vector.tensor_mul(avn3[:, :, ho:ho + 64], pavr[:, :, 1:65], rcp_b)
                    else:
                        for qc in range(8):
                            nc.vector.tensor_scalar_mul(
                                avn3[:, qc:qc + 1, ho:ho + 64],
                                pavr[:, qc:qc + 1, 1:65], rcp[:, qc:qc + 1])
                    if u == 0:
                        dbg("dbg_avn0", avn)
                # pair complete: transpose back to head-dim-major layout
                aT3 = aT[hc].rearrange("p (tc t) -> p tc t", t=128)
                if TRANSPOSE_3D:
                    nc.sync.dma_start_transpose(
                        aT3[:, qh * 8:(qh + 1) * 8, :], avn)
                else:
                    for tcn in range(8):
                        nc.sync.dma_start_transpose(
                            aT[hc][:, qh * 1024 + tcn * 128:qh * 1024 + (tcn + 1) * 128],
                            avn[:, tcn * 128:(tcn + 1) * 128])
        dbg("dbg_aT0", aT[0])

        # ---------------- tail: second half of projection ----------------
        # th2 prerun: open all 8 ec-groups and run their hcc 0-2 matmuls
        # while the last pair's normalize + transpose completes; the hcc3
        # matmul (stop) lands right after aT[3] arrives. Keeps PE busy with
        # no p-state reset across the transpose latency.
        tsl2 = slice(2 * 512, 3 * 512)
        tpools = [psS, psS, psF, psAV]
        t2 = [tpools[j].tile([128, 1024], F32, name=f"tt{j}",
                             tag="pav" if tpools[j] is psAV else "s")
              for j in range(4)]
        for j in range(4):
            for half in range(2):
                ec = 2 * j + half
                sl = slice(half * 512, (half + 1) * 512)
                for i in range(3):
                    nc.tensor.matmul(
                        t2[j][:, sl], wp_t[:, i * 1024 + ec * 128:i * 1024 + (ec + 1) * 128],
                        aT[i][:, tsl2], start=(i == 0), stop=False)
        for j in range(4):
            for half in range(2):
                ec = 2 * j + half
                sl = slice(half * 512, (half + 1) * 512)
                nc.tensor.matmul(
                    t2[j][:, sl], wp_t[:, 3 * 1024 + ec * 128:3 * 1024 + (ec + 1) * 128],
                    aT[3][:, tsl2], start=False, stop=True)
                osb = osbp.tile([128, 512], F32, name=f"osb{ec}_t2", tag="osb")
                nc.vector.tensor_scalar_add(osb, t2[j][:, sl], bpr_t[:, ec:ec + 1])
                nc.sync.dma_start(out[ec * 128:(ec + 1) * 128, tsl2], osb)
        th3_pools = [psS, psS, psF]
        for i, ec in enumerate(range(8)):
            emit_proj(ec, 3, th3_pools[i % 3])


def _input_specs():
    # name -> (shape, dtype_str)
    return {
        "xT": ((C, T), "bf16"), "wqkv": ((C, 3 * C // G), "bf16"),
        "wproj": ((C // G, C), "bf16"),
        "bqk": ((128, 8), "f32"), "bv": ((128, 520), "f32"),
        "bpr": ((128, 8), "f32"),
        "cosT": ((128, T), "bf16"), "sinT": ((128, T), "bf16"),
        "rope_perm": ((128, 128), "bf16"),
    }


def _build_program():
    import concourse.mybir as mybir
    import concourse.tile as tile
    from concourse import bacc

    nc = bacc.Bacc("TRN2", target_bir_lowering=False, debug=False)
    ins = {}
    for name, (shape, dts) in _input_specs().items():
        dt = mybir.dt.bfloat16 if dts == "bf16" else mybir.dt.float32
        ins[name] = nc.dram_tensor(name, list(shape), dt,
                                   kind="ExternalInput").ap()
    outs = {"out": nc.dram_tensor("out", [C, T], mybir.dt.float32,
                                  kind="ExternalOutput").ap()}
    with tile.TileContext(nc) as tc:
        _attn_body(tc, outs, ins)
    nc.compile()
    return nc


def _core_inputs(core, x, W_qkv, b_qkv, W_proj, b_proj, cosT, sinT, P):
    b, g = divmod(core, 2)
    f32 = np.float32
    bf16 = ml_dtypes.bfloat16
    xT = np.ascontiguousarray(np.asarray(x[b], dtype=f32).T).astype(bf16)
    W_qkv = np.asarray(W_qkv, dtype=f32)
    b_qkv = np.asarray(b_qkv, dtype=f32)
    q = W_qkv[:, g * 512:(g + 1) * 512]
    k = W_qkv[:, C + g * 512:C + (g + 1) * 512]
    v = W_qkv[:, 2 * C + g * 512:2 * C + (g + 1) * 512]
    wqkv = np.ascontiguousarray(np.concatenate([q, k, v], axis=1)).astype(bf16)
    bq = b_qkv[g * 512:(g + 1) * 512]
    bk = b_qkv[C + g * 512:C + (g + 1) * 512]
    bqk = np.ascontiguousarray(
        np.stack([bq[i * 128:(i + 1) * 128] for i in range(4)]
                 + [bk[i * 128:(i + 1) * 128] for i in range(4)], axis=1))
    bvr = b_qkv[2 * C + g * 512:2 * C + (g + 1) * 512].reshape(HG, 64)
    bvg = np.concatenate([np.ones((HG, 1), f32), bvr], axis=1).reshape(-1)  # (520,)
    bv = np.ascontiguousarray(np.tile(bvg[None, :], (128, 1)))
    wproj = np.ascontiguousarray(
        np.asarray(W_proj, dtype=f32)[g * 512:(g + 1) * 512]).astype(bf16)
    if g == 0:
        bpr = np.ascontiguousarray(
            np.asarray(b_proj, dtype=f32).reshape(8, 128).T)
    else:
        bpr = np.zeros((128, 8), dtype=f32)
    return {"xT": xT, "wqkv": wqkv, "wproj": wproj, "bqk": bqk, "bv": bv,
            "bpr": bpr, "cosT": cosT, "sinT": sinT, "rope_perm": P}


def run(x, W_qkv, b_qkv, W_proj, b_proj, trace=False):
    from concourse.bass_utils import run_bass_kernel_spmd

    if "nc" not in _CACHED:
        _CACHED["nc"] = _build_program()
    nc = _CACHED["nc"]

    bf16 = ml_dtypes.bfloat16
    cosT, sinT = _rope_tables()
    cosT = cosT.astype(bf16)
    sinT = sinT.astype(bf16)
    P = _perm_table().astype(bf16)
    in_maps = [_core_inputs(c, x, W_qkv, b_qkv, W_proj, b_proj, cosT, sinT, P)
               for c in range(8)]
    res = run_bass_kernel_spmd(nc, in_maps, core_ids=list(range(8)), trace=trace)
    parts = [np.asarray(r["out"], dtype=np.float32) for r in res.results]
    out = np.stack([(parts[2 * b] + parts[2 * b + 1]).T for b in range(B)], axis=0)
    return np.ascontiguousarray(out), res


def kernel(x, W_qkv, b_qkv, W_proj, b_proj):
    out, _ = run(x, W_qkv, b_qkv, W_proj, b_proj, trace=False)
    return out


# revision 22
# speedup vs baseline: 1.5837x; 1.0309x over previous
"""Multi-head attention (RoPE) Trainium2 Bass kernel — pipelined bf16 version.

Problem: B=4, T=2048, C=1024, H=16, d=64, fp32 in/out, full attention + RoPE.
Sharding: 8 cores = 4 batches x 2 head-groups (8 heads each). Each core
computes its batch's attention for its heads plus the partial (transposed)
output projection; the host sums the two head-group partials per batch and
transposes back.

Design notes (cost-model driven):
- All matmul operands are bf16 (1 cycle/row on PE, half the SBUF/DMA of f32).
- AV uses a token-major dataflow: out[q, 65] = ex_chunk^T @ [ones|v], using
  all 128 output partitions (halves AV PE time vs a 65-partition head-major
  form) and making softmax normalization a per-partition scalar multiply.
  The softmax denominator rides along as column 0 via the ones column of vg.
- Normalized attention output transposes back to head-dim-major via one
  SBUF->SBUF DMA xbar transpose per (head-pair, q-half); PE is not involved.
- One instruction stream software-pipelines everything: QKV chunk
  projections, v-projections and the second-half output projection run as PE
  filler inside the ACT-bound attention stretch so neither PE nor the
  Activation engine (exp) ever starves. RoPE for chunk i-1 is emitted inside
  chunk i's slot so its PE permutation-matmul never waits on DVE.
- Projection is emitted transposed (features on partitions) so its bias is a
  per-partition scalar; the host transposes the final result (untimed).
"""

import numpy as np
import ml_dtypes

B, T, C = 4, 2048, 1024
H, D = 16, 64
G = 2              # head groups (cores per batch)
HG = H // G        # heads per core = 8
CC = C // 128      # 8 contraction chunks
NKC = T // 128     # 16 key chunks
NTB = T // 512     # 4 t-blocks
ROPE_BASE = 10000.0
SCALE = 1.0 / np.sqrt(D)

FUSED_NORM = True      # stride-0 free-dim broadcast of 1/den in one DVE op
TRANSPOSE_3D = True    # one xbar DMA transpose per (pair, q-half)

_CACHED = {}


def _rope_tables():
    inv_freq = 1.0 / (ROPE_BASE ** (np.arange(0, D, 2, dtype=np.float32) / D))
    t = np.arange(T, dtype=np.float32)
    freqs = np.outer(t, inv_freq).astype(np.float32)          # (T, 32)
    emb = np.concatenate([freqs, freqs], axis=-1)             # (T, 64)
    cos = np.cos(emb).T.astype(np.float32)                    # (64, T)
    sin = np.sin(emb).T.astype(np.float32)                    # (64, T)
    cosT = np.concatenate([cos, cos], axis=0)                 # (128, T) two heads/chunk
    sinT = np.concatenate([sin, sin], axis=0)
    return np.ascontiguousarray(cosT), np.ascontiguousarray(sinT)


def _perm_table():
    # rot[d] = sum_s P[s, d] * raw[s] = rotate_half with sign, 2 heads/chunk
    P = np.zeros((128, 128), np.float32)
    for d in range(128):
        blk, dd = divmod(d, D)
        if dd < 32:
            P[blk * D + dd + 32, d] = -1.0
        else:
            P[blk * D + dd - 32, d] = 1.0
    return P


def _attn_body(tc, outs, ins):
    """Tile kernel body. ins/outs are dicts of DRAM APs."""
    import contextlib
    import concourse.bass as bass
    import concourse.mybir as mybir

    nc = tc.nc
    F32 = mybir.dt.float32
    BF16 = mybir.dt.bfloat16
    EXP = mybir.ActivationFunctionType.Exp

    xT = ins["xT"]            # (1024, 2048) bf16  x[b].T
    wqkv = ins["wqkv"]        # (1024, 1536) bf16  [Wq | Wk | Wv] cols for group
    wproj = ins["wproj"]      # (512, 1024) bf16
    bqk = ins["bqk"]          # (128, 8) f32 per-chunk per-partition bias
    bv = ins["bv"]            # (128, 520) f32 broadcast [1|v-bias] per head
    bpr = ins["bpr"]          # (128, 8) f32 proj bias (e-chunk cols; zeros g1)
    cosT_d = ins["cosT"]      # (128, 2048) bf16
    sinT_d = ins["sinT"]      # (128, 2048) bf16
    perm_d = ins["rope_perm"]  # (128, 128) bf16 signed rotate_half permutation
    out = outs["out"]         # (1024, 2048) f32 partial transposed output

    def dbg(name, tile_ap):
        if name in outs:
            nc.sync.dma_start(outs[name].bitcast(tile_ap.dtype), tile_ap)

    ctx = contextlib.ExitStack()
    with ctx:
        pers = ctx.enter_context(tc.tile_pool(name="pers", bufs=1))

        # ---------------- persistent tiles ----------------
        x_t = pers.tile([128, CC * T], BF16, name="x_t", tag="x_t")
        wqk_t = pers.tile([128, CC * 1024], BF16, name="wqk_t", tag="wqk_t")
        wv_t = pers.tile([128, CC * 512], BF16, name="wv_t", tag="wv_t")
        wp_t = pers.tile([128, 4 * 1024], BF16, name="wp_t", tag="wp_t")
        cos_t = pers.tile([128, T], BF16, name="cos_t", tag="cos_t")
        sin_t = pers.tile([128, T], BF16, name="sin_t", tag="sin_t")
        perm_t = pers.tile([128, 128], BF16, name="perm_t", tag="perm_t")
        bqk_t = pers.tile([128, 8], F32, name="bqk_t", tag="bqk_t")
        bv_t = pers.tile([128, 520], F32, name="bv_t", tag="bv_t")
        bpr_t = pers.tile([128, 8], F32, name="bpr_t", tag="bpr_t")
        qk = [pers.tile([128, T], BF16, name=f"qk{j}", tag=f"qk{j}") for j in range(8)]
        vg = [pers.tile([128, HG * 65], BF16, name=f"vg{k}", tag=f"vg{k}") for k in range(NKC)]
        aT = [pers.tile([128, T], BF16, name=f"aT{i}", tag=f"aT{i}") for i in range(4)]

        # ---------------- working pools ----------------
        # PSUM: psS 2x2 banks (scores/exp), psAV 1x2 banks (AV accum),
        # psF 1x2 banks (qkv/v/proj filler groups + rope perm outputs).
        psS = ctx.enter_context(tc.tile_pool(name="psS", bufs=2, space="PSUM"))
        psAV = ctx.enter_context(tc.tile_pool(name="psAV", bufs=1, space="PSUM"))
        psF = ctx.enter_context(tc.tile_pool(name="psF", bufs=1, space="PSUM"))
        expool = ctx.enter_context(tc.tile_pool(name="expool", bufs=6))
        ex1p = ctx.enter_context(tc.tile_pool(name="ex1p", bufs=16))
        rawp = ctx.enter_context(tc.tile_pool(name="rawp", bufs=2))
        tmpp = ctx.enter_context(tc.tile_pool(name="tmpp", bufs=2))
        tmpcp = ctx.enter_context(tc.tile_pool(name="tmpcp", bufs=2))
        denp = ctx.enter_context(tc.tile_pool(name="denp", bufs=2))
        rcpp = ctx.enter_context(tc.tile_pool(name="rcpp", bufs=2))
        avnp = ctx.enter_context(tc.tile_pool(name="avnp", bufs=2))
        osbp = ctx.enter_context(tc.tile_pool(name="osbp", bufs=5))

        # ---------------- input DMAs (no waits; ordered for earliest use) ---
        x3d = x_t.rearrange("p (c t) -> p c t", t=T)
        xTd = xT.rearrange("(c p) t -> p c t", p=128)
        wqk3 = wqk_t.rearrange("p (c e) -> p c e", e=1024)

        def w_slice(jc):
            # per-chunk 128-col slice of [Wq|Wk] for q/k chunk jc
            col0 = (jc % 4) * 128 + (512 if jc >= 4 else 0)
            nc.sync.dma_start(
                wqk3[:, :, col0:col0 + 128],
                wqkv[:, col0:col0 + 128].rearrange("(c p) e -> p c e", p=128))

        nc.sync.dma_start(perm_t, perm_d)
        w_slice(0)
        nc.sync.dma_start(x3d[:, :, 0:512], xTd[:, :, 0:512])
        nc.sync.dma_start(bqk_t, bqk)
        nc.sync.dma_start(cos_t, cosT_d)
        nc.sync.dma_start(sin_t, sinT_d)
        w_slice(4)
        nc.sync.dma_start(x3d[:, :, 512:1024], xTd[:, :, 512:1024])
        nc.sync.dma_start(wv_t.rearrange("p (c e) -> p c e", e=512),
                          wqkv[:, 1024:1536].rearrange("(c p) e -> p c e", p=128))
        nc.sync.dma_start(bv_t, bv)
        nc.sync.dma_start(x3d[:, :, 1024:1536], xTd[:, :, 1024:1536])
        nc.sync.dma_start(x3d[:, :, 1536:2048], xTd[:, :, 1536:2048])
        for jc in (1, 5, 2, 6, 3, 7):
            w_slice(jc)
        nc.sync.dma_start(bpr_t, bpr)
        nc.sync.dma_start(wp_t.rearrange("p (i e) -> p i e", e=1024),
                          wproj.rearrange("(i p) e -> p i e", p=128))

        # PE clock warmup: keep the tensor engine continuously busy from the
        # moment perm_t lands until the first real slot's inputs arrive, so
        # the p-state model reaches full clock before real work dispatches.
        warm = psF.tile([128, 1024], F32, name="warm", tag="s")
        for i in range(40):
            nc.tensor.matmul(warm[:, 0:128], perm_t, perm_t, start=True, stop=True)

        # ---------------- emitters ----------------
        pend = {"rope": None}  # (jc, tb, raw) awaiting perm-matmul + combine

        def emit_rope(ps_half):
            """Emit pending RoPE combine: perm-matmul into ps_half (psum
            (128,512) f32 slice), then DVE combine into qk[jc]."""
            jc, tb, raw = pend["rope"]
            pend["rope"] = None
            tsl = slice(tb * 512, (tb + 1) * 512)
            nc.tensor.matmul(ps_half, perm_t, raw, start=True, stop=True)
            tmp = tmpp.tile([128, 512], F32, name=f"tm{jc}_{tb}", tag="tmp")
            nc.vector.tensor_mul(tmp, ps_half, sin_t[:, tsl])
            tmpc = tmpcp.tile([128, 512], F32, name=f"tc{jc}_{tb}", tag="tmpc")
            nc.vector.tensor_mul(tmpc, raw, cos_t[:, tsl])
            nc.vector.tensor_add(qk[jc][:, tsl], tmp, tmpc)

        def emit_qk_slot(jc, tb, pool):
            """8 projection matmuls for q/k chunk jc, t-block tb, plus the
            RoPE combine of the previously emitted chunk."""
            col0 = (jc % 4) * 128 + (512 if jc >= 4 else 0)
            tsl = slice(tb * 512, (tb + 1) * 512)
            ps = pool.tile([128, 1024], F32, name=f"psq{jc}_{tb}", tag="s")
            if pend["rope"] is not None:
                emit_rope(ps[:, 512:1024])
            for c in range(CC):
                nc.tensor.matmul(
                    ps[:, 0:512], wqk_t[:, c * 1024 + col0:c * 1024 + col0 + 128],
                    x_t[:, c * T + tb * 512:c * T + (tb + 1) * 512],
                    start=(c == 0), stop=(c == CC - 1))
            raw = rawp.tile([128, 512], BF16, name=f"raw{jc}_{tb}", tag="raw")
            nc.vector.tensor_scalar_add(raw, ps[:, 0:512], bqk_t[:, jc:jc + 1])
            pend["rope"] = (jc, tb, raw)

        def emit_rope_flush(pool):
            ps = pool.tile([128, 1024], F32, name="psflush", tag="s")
            emit_rope(ps[:, 512:1024])

        def emit_v(kc, pool):
            """v for token chunk kc -> vg[kc] = [1|v] per head, bf16."""
            ps = pool.tile([128, 1024], F32, name=f"psv{kc}", tag="s")
            for c in range(CC):
                nc.tensor.matmul(
                    ps[:, 0:512], x_t[:, c * T + kc * 128:c * T + (kc + 1) * 128],
                    wv_t[:, c * 512:(c + 1) * 512],
                    start=(c == 0), stop=(c == CC - 1))
            vv = vg[kc].rearrange("p (g w) -> p g w", w=65)
            bvv = bv_t.rearrange("p (g w) -> p g w", w=65)
            psg = ps[:, 0:512].rearrange("p (g d) -> p g d", d=64)
            nc.vector.tensor_add(vv[:, :, 1:65], psg, bvv[:, :, 1:65])
            nc.vector.tensor_copy(vv[:, :, 0:1], bvv[:, :, 0:1])

        def emit_proj(ec, th, pool):
            """transposed proj: out rows = e-chunk ec, cols = tok block th."""
            tsl = slice(th * 512, (th + 1) * 512)
            ps = pool.tile([128, 1024], F32, name=f"psp{ec}_{th}", tag="s")
            for i in range(4):
                nc.tensor.matmul(
                    ps[:, 0:512], wp_t[:, i * 1024 + ec * 128:i * 1024 + (ec + 1) * 128],
                    aT[i][:, tsl], start=(i == 0), stop=(i == 3))
            osb = osbp.tile([128, 512], F32, name=f"osb{ec}_{th}", tag="osb")
            nc.vector.tensor_scalar_add(osb, ps[:, 0:512], bpr_t[:, ec:ec + 1])
            nc.sync.dma_start(out[ec * 128:(ec + 1) * 128, tsl], osb)

        def emit_av(pav, ex, kc, h):
            # start=True zeroes the whole PSUM bank, so only the first group
            # of each bank (qc 0 and 4) may set it; the bank-wide zero covers
            # the other interleaved accumulation groups' regions.
            mv = vg[kc][:, h * 65:(h + 1) * 65]
            for qc in range(8):
                nc.tensor.matmul(
                    pav[:, qc * 128:qc * 128 + 65],
                    ex[:, qc * 128:(qc + 1) * 128], mv,
                    start=(kc == 0 and qc % 4 == 0), stop=(kc == NKC - 1))

        def norm(pav, p, avn3):
            """normalize: avn[:, tc, p*64+d] = pav[:, tc, 1+d] / pav[:, tc, 0]"""
            ho = p * 64
            pavr = pav.rearrange("p (qc w) -> p qc w", w=128)
            den = denp.tile([128, 8], F32, name=f"den{id(pav)}_{p}", tag="den")
            den3 = den.rearrange("p (a b) -> p a b", b=1)
            nc.vector.tensor_copy(den3, pavr[:, :, 0:1])
            rcp = rcpp.tile([128, 8], F32, name=f"rcp{id(pav)}_{p}", tag="rcp")
            nc.vector.reciprocal(rcp, den)
            if FUSED_NORM:
                rcp_b = bass.AP(tensor=rcp.tensor, offset=rcp.offset,
                                ap=[list(rcp.ap[0]), [1, 8], [0, 64]])
                nc.vector.tensor_mul(avn3[:, :, ho:ho + 64], pavr[:, :, 1:65], rcp_b)
            else:
                for qc in range(8):
                    nc.vector.tensor_scalar_mul(
                        avn3[:, qc:qc + 1, ho:ho + 64],
                        pavr[:, qc:qc + 1, 1:65], rcp[:, qc:qc + 1])

        def emit_T(hc, qh, avn):
            aT3 = aT[hc].rearrange("p (tc t) -> p tc t", t=128)
            if TRANSPOSE_3D:
                nc.sync.dma_start_transpose(aT3[:, qh * 8:(qh + 1) * 8, :], avn)
            else:
                for tcn in range(8):
                    nc.sync.dma_start_transpose(
                        aT[hc][:, qh * 1024 + tcn * 128:qh * 1024 + (tcn + 1) * 128],
                        avn[:, tcn * 128:(tcn + 1) * 128])

        def emit_S_E(h, qh, kc, kt, qt):
            ho = (h % 2) * 64
            s = psS.tile([128, 1024], F32, name=f"s{h}_{qh}_{kc}", tag="s")
            ksl = slice(kc * 128, (kc + 1) * 128)
            for qq in range(2):
                qsl = slice(qh * 1024 + qq * 512, qh * 1024 + (qq + 1) * 512)
                nc.tensor.matmul(
                    s[:, qq * 512:(qq + 1) * 512],
                    kt[ho:ho + 64, ksl], qt[ho:ho + 64, qsl],
                    start=True, stop=True)
            pool = ex1p if (h, qh) == (1, 0) else expool
            tag = "ex1" if (h, qh) == (1, 0) else "ex"
            ex = pool.tile([128, 1024], BF16, name=f"ex{h}_{qh}_{kc}", tag=tag)
            nc.scalar.activation(ex, s, EXP, bias=0.0, scale=float(SCALE))
            return ex

        # ======== fused wall: units (h0,qh0)+(h1,qh0) share one S/E stream ==
        # All v-chunks and the remaining pair-0 qk slots run here as filler;
        # h1's AV is deferred into unit (h0,qh1) so the Activation engine gets
        # two units of exp supply while PE chews through the projection wall.
        emit_qk_slot(0, 0, psS)
        emit_qk_slot(4, 0, psS)
        emit_qk_slot(0, 1, psS)
        emit_qk_slot(4, 1, psS)
        emit_rope_flush(psS)
        wall_slots = {1: (4, 2), 3: (0, 2), 5: (4, 3), 7: (0, 3)}
        pav0 = psAV.tile([128, 1024], F32, name="pav0", tag="pav")
        ex0s = [None] * NKC
        ex1s = [None] * NKC
        avn00 = avnp.tile([128, 1024], BF16, name="avn00", tag="avn")
        avn00_3 = avn00.rearrange("p (tc w) -> p tc w", w=128)
        for kc in range(NKC):
            if kc in wall_slots:
                emit_qk_slot(*wall_slots[kc], psS)
            elif kc == 9:
                emit_rope_flush(psS)
            ex0s[kc] = emit_S_E(0, 0, kc, qk[4], qk[0])
            ex1s[kc] = emit_S_E(1, 0, kc, qk[4], qk[0])
            emit_v(kc, psF)
            if kc >= 2:
                emit_av(pav0, ex0s[kc - 2], kc - 2, 0)
        emit_av(pav0, ex0s[14], 14, 0)
        emit_av(pav0, ex0s[15], 15, 0)
        dbg("dbg_ex0", ex0s[0])
        norm(pav0, 0, avn00_3)
        dbg("dbg_avn0", avn00)

        pav1_box = [None]

        def a1_item(lo, hi):
            def f():
                if pav1_box[0] is None:
                    pav1_box[0] = psAV.tile([128, 1024], F32, name="pav1", tag="pav")
                for kc2 in range(lo, hi):
                    emit_av(pav1_box[0], ex1s[kc2], kc2, 1)
            return f

        def norm1_item():
            norm(pav1_box[0], 1, avn00_3)
            emit_T(0, 0, avn00)

        # filler items per unit index (u = 4*hc + 2*qh + p)
        def qk_item(jc, tb):
            return lambda: emit_qk_slot(jc, tb, psF)

        flush = lambda: emit_rope_flush(psF)
        unit_fill = {u: [] for u in range(16)}
        unit_fill[2] = [a1_item(0, 8), a1_item(8, 16), norm1_item,
                        qk_item(1, 0), qk_item(1, 1), qk_item(5, 0)]
        unit_fill[3] = [qk_item(1, 2), qk_item(5, 1), qk_item(1, 3)]
        unit_fill[4] = [qk_item(5, 2), qk_item(5, 3), flush]
        unit_fill[5] = [qk_item(2, 0), qk_item(2, 1), qk_item(6, 0)]
        unit_fill[6] = [qk_item(2, 2), qk_item(6, 1), qk_item(2, 3)]
        unit_fill[7] = [qk_item(6, 2), qk_item(6, 3), flush]
        unit_fill[8] = [qk_item(3, 0), qk_item(3, 1), qk_item(7, 0)]
        unit_fill[9] = [qk_item(3, 2), qk_item(7, 1), qk_item(3, 3)]
        unit_fill[10] = [qk_item(7, 2), qk_item(7, 3), flush]
        unit_fill[14] = [(lambda ec: (lambda: emit_proj(ec, 0, psF)))(ec) for ec in range(8)]
        unit_fill[15] = [(lambda ec: (lambda: emit_proj(ec, 1, psF)))(ec) for ec in range(8)]

        # ---------------- remaining attention units ----------------
        for hc in range(4):
            for qh in range(2):
                if hc == 0 and qh == 0:
                    continue  # handled by the fused wall above
                avn = avnp.tile([128, 1024], BF16, name=f"avn{hc}_{qh}", tag="avn")
                avn3 = avn.rearrange("p (tc w) -> p tc w", w=128)
                for p in range(2):
                    h = 2 * hc + p
                    u = 4 * hc + 2 * qh + p
                    qt = qk[hc]
                    kt = qk[4 + hc]
                    fills = unit_fill[u]
                    nfill = len(fills)
                    step = max(1, NKC // nfill) if nfill else NKC + 1
                    av_lag = 5 if u == 2 else 1
                    # pav is allocated at first use so psAV slot rotation
                    # follows emission order (pav1 is created inside u2's
                    # fillers, before this unit's first AV matmul).
                    pav = None
                    exs = [None] * NKC
                    fi = 0
                    for kc in range(NKC):
                        exs[kc] = emit_S_E(h, qh, kc, kt, qt)
                        if fi < nfill and kc % step == 0:
                            fills[fi]()
                            fi += 1
                        if kc >= av_lag:
                            if pav is None:
                                pav = psAV.tile([128, 1024], F32,
                                                name=f"pav{h}_{qh}", tag="pav")
                            emit_av(pav, exs[kc - av_lag], kc - av_lag, h)
                    while fi < nfill:
                        fills[fi]()
                        fi += 1
                    for kc in range(NKC - av_lag, NKC):
                        emit_av(pav, exs[kc], kc, h)
                    norm(pav, p, avn3)
                emit_T(hc, qh, avn)
        dbg("dbg_aT0", aT[0])

        # ---------------- tail: second half of projection ----------------
        # th2 prerun: open all 8 ec-groups and run their hcc 0-2 matmuls
        # while the last pair's normalize + transpose completes; the hcc3
        # matmul (stop) lands right after aT[3] arrives. Keeps PE busy with
        # no p-state reset across the transpose latency.
        tsl2 = slice(2 * 512, 3 * 512)
        tpools = [psS, psS, psF, psAV]
        t2 = [tpools[j].tile([128, 1024], F32, name=f"tt{j}",
                             tag="pav" if tpools[j] is psAV else "s")
              for j in range(4)]
        for j in range(4):
            for half in range(2):
                ec = 2 * j + half
                sl = slice(half * 512, (half + 1) * 512)
                for i in range(3):
                    nc.tensor.matmul(
                        t2[j][:, sl], wp_t[:, i * 1024 + ec * 128:i * 1024 + (ec + 1) * 128],
                        aT[i][:, tsl2], start=(i == 0), stop=False)
        for j in range(4):
            for half in range(2):
                ec = 2 * j + half
                sl = slice(half * 512, (half + 1) * 512)
                nc.tensor.matmul(
                    t2[j][:, sl], wp_t[:, 3 * 1024 + ec * 128:3 * 1024 + (ec + 1) * 128],
                    aT[3][:, tsl2], start=False, stop=True)
                osb = osbp.tile([128, 512], F32, name=f"osb{ec}_t2", tag="osb")
                nc.vector.tensor_scalar_add(osb, t2[j][:, sl], bpr_t[:, ec:ec + 1])
                nc.sync.dma_start(out[ec * 128:(ec + 1) * 128, tsl2], osb)
        th3_pools = [psS, psS, psF]
        for i, ec in enumerate(range(8)):
            emit_proj(ec, 3, th3_pools[i % 3])


def _input_specs():
    # name -> (shape, dtype_str)
    return {
        "xT": ((C, T), "bf16"), "wqkv": ((C, 3 * C // G), "bf16"),
        "wproj": ((C // G, C), "bf16"),
        "bqk": ((128, 8), "f32"), "bv": ((128, 520), "f32"),
        "bpr": ((128, 8), "f32"),
        "cosT": ((128, T), "bf16"), "sinT": ((128, T), "bf16"),
        "rope_perm": ((128, 128), "bf16"),
    }


def _build_program():
    import concourse.mybir as mybir
    import concourse.tile as tile
    from concourse import bacc

    nc = bacc.Bacc("TRN2", target_bir_lowering=False, debug=False)
    ins = {}
    for name, (shape, dts) in _input_specs().items():
        dt = mybir.dt.bfloat16 if dts == "bf16" else mybir.dt.float32
        ins[name] = nc.dram_tensor(name, list(shape), dt,
                                   kind="ExternalInput").ap()
    outs = {"out": nc.dram_tensor("out", [C, T], mybir.dt.float32,
                                  kind="ExternalOutput").ap()}
    with tile.TileContext(nc) as tc:
        _attn_body(tc, outs, ins)
    nc.compile()
    return nc


def _core_inputs(core, x, W_qkv, b_qkv, W_proj, b_proj, cosT, sinT, P):
    b, g = divmod(core, 2)
    f32 = np.float32
    bf16 = ml_dtypes.bfloat16
    xT = np.ascontiguousarray(np.asarray(x[b], dtype=f32).T).astype(bf16)
    W_qkv = np.asarray(W_qkv, dtype=f32)
    b_qkv = np.asarray(b_qkv, dtype=f32)
    q = W_qkv[:, g * 512:(g + 1) * 512]
    k = W_qkv[:, C + g * 512:C + (g + 1) * 512]
    v = W_qkv[:, 2 * C + g * 512:2 * C + (g + 1) * 512]
    wqkv = np.ascontiguousarray(np.concatenate([q, k, v], axis=1)).astype(bf16)
    bq = b_qkv[g * 512:(g + 1) * 512]
    bk = b_qkv[C + g * 512:C + (g + 1) * 512]
    bqk = np.ascontiguousarray(
        np.stack([bq[i * 128:(i + 1) * 128] for i in range(4)]
                 + [bk[i * 128:(i + 1) * 128] for i in range(4)], axis=1))
    bvr = b_qkv[2 * C + g * 512:2 * C + (g + 1) * 512].reshape(HG, 64)
    bvg = np.concatenate([np.ones((HG, 1), f32), bvr], axis=1).reshape(-1)  # (520,)
    bv = np.ascontiguousarray(np.tile(bvg[None, :], (128, 1)))
    wproj = np.ascontiguousarray(
        np.asarray(W_proj, dtype=f32)[g * 512:(g + 1) * 512]).astype(bf16)
    if g == 0:
        bpr = np.ascontiguousarray(
            np.asarray(b_proj, dtype=f32).reshape(8, 128).T)
    else:
        bpr = np.zeros((128, 8), dtype=f32)
    return {"xT": xT, "wqkv": wqkv, "wproj": wproj, "bqk": bqk, "bv": bv,
            "bpr": bpr, "cosT": cosT, "sinT": sinT, "rope_perm": P}


def run(x, W_qkv, b_qkv, W_proj, b_proj, trace=False):
    from concourse.bass_utils import run_bass_kernel_spmd

    if "nc" not in _CACHED:
        _CACHED["nc"] = _build_program()
    nc = _CACHED["nc"]

    bf16 = ml_dtypes.bfloat16
    cosT, sinT = _rope_tables()
    cosT = cosT.astype(bf16)
    sinT = sinT.astype(bf16)
    P = _perm_table().astype(bf16)
    in_maps = [_core_inputs(c, x, W_qkv, b_qkv, W_proj, b_proj, cosT, sinT, P)
               for c in range(8)]
    res = run_bass_kernel_spmd(nc, in_maps, core_ids=list(range(8)), trace=trace)
    parts = [np.asarray(r["out"], dtype=np.float32) for r in res.results]
    out = np.stack([(parts[2 * b] + parts[2 * b + 1]).T for b in range(B)], axis=0)
    return np.ascontiguousarray(out), res


def kernel(x, W_qkv, b_qkv, W_proj, b_proj):
    out, _ = run(x, W_qkv, b_qkv, W_proj, b_proj, trace=False)
    return out


# revision 25
# speedup vs baseline: 1.6019x; 1.0115x over previous
"""Multi-head attention (RoPE) Trainium2 Bass kernel — pipelined bf16 version.

Problem: B=4, T=2048, C=1024, H=16, d=64, fp32 in/out, full attention + RoPE.
Sharding: 8 cores = 4 batches x 2 head-groups (8 heads each). Each core
computes its batch's attention for its heads plus the partial (transposed)
output projection; the host sums the two head-group partials per batch and
transposes back.

Design notes (cost-model driven):
- All matmul operands are bf16 (1 cycle/row on PE, half the SBUF/DMA of f32).
- AV uses a token-major dataflow: out[q, 65] = ex_chunk^T @ [ones|v], using
  all 128 output partitions (halves AV PE time vs a 65-partition head-major
  form) and making softmax normalization a per-partition scalar multiply.
  The softmax denominator rides along as column 0 via the ones column of vg.
- Normalized attention output transposes back to head-dim-major via one
  SBUF->SBUF DMA xbar transpose per (head-pair, q-half); PE is not involved.
- One instruction stream software-pipelines everything: QKV chunk
  projections, v-projections and the second-half output projection run as PE
  filler inside the ACT-bound attention stretch so neither PE nor the
  Activation engine (exp) ever starves. RoPE for chunk i-1 is emitted inside
  chunk i's slot so its PE permutation-matmul never waits on DVE.
- Projection is emitted transposed (features on partitions) so its bias is a
  per-partition scalar; the host transposes the final result (untimed).
"""

import numpy as np
import ml_dtypes

B, T, C = 4, 2048, 1024
H, D = 16, 64
G = 2              # head groups (cores per batch)
HG = H // G        # heads per core = 8
CC = C // 128      # 8 contraction chunks
NKC = T // 128     # 16 key chunks
NTB = T // 512     # 4 t-blocks
ROPE_BASE = 10000.0
SCALE = 1.0 / np.sqrt(D)

FUSED_NORM = True      # stride-0 free-dim broadcast of 1/den in one DVE op
TRANSPOSE_3D = True    # one xbar DMA transpose per (pair, q-half)

_CACHED = {}


def _rope_tables():
    inv_freq = 1.0 / (ROPE_BASE ** (np.arange(0, D, 2, dtype=np.float32) / D))
    t = np.arange(T, dtype=np.float32)
    freqs = np.outer(t, inv_freq).astype(np.float32)          # (T, 32)
    emb = np.concatenate([freqs, freqs], axis=-1)             # (T, 64)
    cos = np.cos(emb).T.astype(np.float32)                    # (64, T)
    sin = np.sin(emb).T.astype(np.float32)                    # (64, T)
    cosT = np.concatenate([cos, cos], axis=0)                 # (128, T) two heads/chunk
    sinT = np.concatenate([sin, sin], axis=0)
    return np.ascontiguousarray(cosT), np.ascontiguousarray(sinT)


def _perm_table():
    # rot[d] = sum_s P[s, d] * raw[s] = rotate_half with sign, 2 heads/chunk
    P = np.zeros((128, 128), np.float32)
    for d in range(128):
        blk, dd = divmod(d, D)
        if dd < 32:
            P[blk * D + dd + 32, d] = -1.0
        else:
            P[blk * D + dd - 32, d] = 1.0
    return P


def _attn_body(tc, outs, ins):
    """Tile kernel body. ins/outs are dicts of DRAM APs."""
    import contextlib
    import concourse.bass as bass
    import concourse.mybir as mybir

    nc = tc.nc
    F32 = mybir.dt.float32
    BF16 = mybir.dt.bfloat16
    EXP = mybir.ActivationFunctionType.Exp

    xT = ins["xT"]            # (1024, 2048) bf16  x[b].T
    wqkv = ins["wqkv"]        # (1024, 1536) bf16  [Wq | Wk | Wv] cols for group
    wproj = ins["wproj"]      # (512, 1024) bf16
    bqk = ins["bqk"]          # (128, 8) f32 per-chunk per-partition bias
    bv = ins["bv"]            # (128, 520) f32 broadcast [1|v-bias] per head
    bpr = ins["bpr"]          # (128, 8) f32 proj bias (e-chunk cols; zeros g1)
    cosT_d = ins["cosT"]      # (128, 2048) bf16
    sinT_d = ins["sinT"]      # (128, 2048) bf16
    perm_d = ins["rope_perm"]  # (128, 128) bf16 signed rotate_half permutation
    out = outs["out"]         # (1024, 2048) f32 partial transposed output

    def dbg(name, tile_ap):
        if name in outs:
            nc.sync.dma_start(outs[name].bitcast(tile_ap.dtype), tile_ap)

    ctx = contextlib.ExitStack()
    with ctx:
        pers = ctx.enter_context(tc.tile_pool(name="pers", bufs=1))

        # ---------------- persistent tiles ----------------
        x_t = pers.tile([128, CC * T], BF16, name="x_t", tag="x_t")
        wqk_t = pers.tile([128, CC * 1024], BF16, name="wqk_t", tag="wqk_t")
        wv_t = pers.tile([128, CC * 512], BF16, name="wv_t", tag="wv_t")
        wp_t = pers.tile([128, 4 * 1024], BF16, name="wp_t", tag="wp_t")
        cos_t = pers.tile([128, T], BF16, name="cos_t", tag="cos_t")
        sin_t = pers.tile([128, T], BF16, name="sin_t", tag="sin_t")
        perm_t = pers.tile([128, 128], BF16, name="perm_t", tag="perm_t")
        bqk_t = pers.tile([128, 8], F32, name="bqk_t", tag="bqk_t")
        bv_t = pers.tile([128, 520], F32, name="bv_t", tag="bv_t")
        bpr_t = pers.tile([128, 8], F32, name="bpr_t", tag="bpr_t")
        qk = [pers.tile([128, T], BF16, name=f"qk{j}", tag=f"qk{j}") for j in range(8)]
        vg = [pers.tile([128, HG * 65], BF16, name=f"vg{k}", tag=f"vg{k}") for k in range(NKC)]
        aT = [pers.tile([128, T], BF16, name=f"aT{i}", tag=f"aT{i}") for i in range(4)]

        # ---------------- working pools ----------------
        # PSUM: psS 2x2 banks (scores/exp), psAV 1x2 banks (AV accum),
        # psF 1x2 banks (qkv/v/proj filler groups + rope perm outputs).
        psS = ctx.enter_context(tc.tile_pool(name="psS", bufs=2, space="PSUM"))
        psAV = ctx.enter_context(tc.tile_pool(name="psAV", bufs=1, space="PSUM"))
        psF = ctx.enter_context(tc.tile_pool(name="psF", bufs=1, space="PSUM"))
        expool = ctx.enter_context(tc.tile_pool(name="expool", bufs=6))
        ex1p = ctx.enter_context(tc.tile_pool(name="ex1p", bufs=16))
        rawp = ctx.enter_context(tc.tile_pool(name="rawp", bufs=2))
        tmpp = ctx.enter_context(tc.tile_pool(name="tmpp", bufs=2))
        tmpcp = ctx.enter_context(tc.tile_pool(name="tmpcp", bufs=2))
        denp = ctx.enter_context(tc.tile_pool(name="denp", bufs=2))
        rcpp = ctx.enter_context(tc.tile_pool(name="rcpp", bufs=2))
        avnp = ctx.enter_context(tc.tile_pool(name="avnp", bufs=2))
        osbp = ctx.enter_context(tc.tile_pool(name="osbp", bufs=5))

        # ---------------- input DMAs (no waits; ordered for earliest use) ---
        x3d = x_t.rearrange("p (c t) -> p c t", t=T)
        xTd = xT.rearrange("(c p) t -> p c t", p=128)
        wqk3 = wqk_t.rearrange("p (c e) -> p c e", e=1024)

        def w_slice(jc):
            # per-chunk 128-col slice of [Wq|Wk] for q/k chunk jc
            col0 = (jc % 4) * 128 + (512 if jc >= 4 else 0)
            nc.sync.dma_start(
                wqk3[:, :, col0:col0 + 128],
                wqkv[:, col0:col0 + 128].rearrange("(c p) e -> p c e", p=128))

        nc.sync.dma_start(perm_t, perm_d)
        w_slice(0)
        nc.sync.dma_start(x3d[:, :, 0:512], xTd[:, :, 0:512])
        nc.sync.dma_start(bqk_t, bqk)
        nc.sync.dma_start(cos_t, cosT_d)
        nc.sync.dma_start(sin_t, sinT_d)
        w_slice(4)
        nc.sync.dma_start(x3d[:, :, 512:1024], xTd[:, :, 512:1024])
        nc.sync.dma_start(wv_t.rearrange("p (c e) -> p c e", e=512),
                          wqkv[:, 1024:1536].rearrange("(c p) e -> p c e", p=128))
        nc.sync.dma_start(bv_t, bv)
        nc.sync.dma_start(x3d[:, :, 1024:1536], xTd[:, :, 1024:1536])
        nc.sync.dma_start(x3d[:, :, 1536:2048], xTd[:, :, 1536:2048])
        for jc in (1, 5, 2, 6, 3, 7):
            w_slice(jc)
        nc.sync.dma_start(bpr_t, bpr)
        nc.sync.dma_start(wp_t.rearrange("p (i e) -> p i e", e=1024),
                          wproj.rearrange("(i p) e -> p i e", p=128))

        # PE clock warmup: keep the tensor engine continuously busy from the
        # moment perm_t lands until the first real slot's inputs arrive, so
        # the p-state model reaches full clock before real work dispatches.
        warm = psF.tile([128, 1024], F32, name="warm", tag="s")
        for i in range(40):
            nc.tensor.matmul(warm[:, 0:128], perm_t, perm_t, start=True, stop=True)

        # ---------------- emitters ----------------
        pend = {"rope": None}  # (jc, tb, raw) awaiting perm-matmul + combine

        def emit_rope(ps_half):
            """Emit pending RoPE combine: perm-matmul into ps_half (psum
            (128,512) f32 slice), then DVE combine into qk[jc]."""
            jc, tb, raw = pend["rope"]
            pend["rope"] = None
            tsl = slice(tb * 512, (tb + 1) * 512)
            nc.tensor.matmul(ps_half, perm_t, raw, start=True, stop=True)
            tmp = tmpp.tile([128, 512], F32, name=f"tm{jc}_{tb}", tag="tmp")
            nc.vector.tensor_mul(tmp, ps_half, sin_t[:, tsl])
            tmpc = tmpcp.tile([128, 512], F32, name=f"tc{jc}_{tb}", tag="tmpc")
            nc.vector.tensor_mul(tmpc, raw, cos_t[:, tsl])
            nc.vector.tensor_add(qk[jc][:, tsl], tmp, tmpc)

        def emit_qk_slot(jc, tb, pool):
            """8 projection matmuls for q/k chunk jc, t-block tb, plus the
            RoPE combine of the previously emitted chunk."""
            col0 = (jc % 4) * 128 + (512 if jc >= 4 else 0)
            tsl = slice(tb * 512, (tb + 1) * 512)
            ps = pool.tile([128, 1024], F32, name=f"psq{jc}_{tb}", tag="s")
            if pend["rope"] is not None:
                emit_rope(ps[:, 512:1024])
            for c in range(CC):
                nc.tensor.matmul(
                    ps[:, 0:512], wqk_t[:, c * 1024 + col0:c * 1024 + col0 + 128],
                    x_t[:, c * T + tb * 512:c * T + (tb + 1) * 512],
                    start=(c == 0), stop=(c == CC - 1))
            raw = rawp.tile([128, 512], BF16, name=f"raw{jc}_{tb}", tag="raw")
            nc.vector.tensor_scalar_add(raw, ps[:, 0:512], bqk_t[:, jc:jc + 1])
            pend["rope"] = (jc, tb, raw)

        def emit_rope_flush(pool):
            ps = pool.tile([128, 1024], F32, name="psflush", tag="s")
            emit_rope(ps[:, 512:1024])

        def emit_v(kc, pool):
            """v for token chunk kc -> vg[kc] = [1|v] per head, bf16."""
            ps = pool.tile([128, 1024], F32, name=f"psv{kc}", tag="s")
            for c in range(CC):
                nc.tensor.matmul(
                    ps[:, 0:512], x_t[:, c * T + kc * 128:c * T + (kc + 1) * 128],
                    wv_t[:, c * 512:(c + 1) * 512],
                    start=(c == 0), stop=(c == CC - 1))
            vv = vg[kc].rearrange("p (g w) -> p g w", w=65)
            bvv = bv_t.rearrange("p (g w) -> p g w", w=65)
            psg = ps[:, 0:512].rearrange("p (g d) -> p g d", d=64)
            nc.vector.tensor_add(vv[:, :, 1:65], psg, bvv[:, :, 1:65])
            nc.vector.tensor_copy(vv[:, :, 0:1], bvv[:, :, 0:1])

        def emit_proj(ec, th, pool):
            """transposed proj: out rows = e-chunk ec, cols = tok block th."""
            tsl = slice(th * 512, (th + 1) * 512)
            ps = pool.tile([128, 1024], F32, name=f"psp{ec}_{th}", tag="s")
            for i in range(4):
                nc.tensor.matmul(
                    ps[:, 0:512], wp_t[:, i * 1024 + ec * 128:i * 1024 + (ec + 1) * 128],
                    aT[i][:, tsl], start=(i == 0), stop=(i == 3))
            osb = osbp.tile([128, 512], F32, name=f"osb{ec}_{th}", tag="osb")
            nc.vector.tensor_scalar_add(osb, ps[:, 0:512], bpr_t[:, ec:ec + 1])
            nc.sync.dma_start(out[ec * 128:(ec + 1) * 128, tsl], osb)

        def emit_av(pav, ex, kc, h):
            # start=True zeroes the whole PSUM bank, so only the first group
            # of each bank (qc 0 and 4) may set it; the bank-wide zero covers
            # the other interleaved accumulation groups' regions.
            mv = vg[kc][:, h * 65:(h + 1) * 65]
            for qc in range(8):
                nc.tensor.matmul(
                    pav[:, qc * 128:qc * 128 + 65],
                    ex[:, qc * 128:(qc + 1) * 128], mv,
                    start=(kc == 0 and qc % 4 == 0), stop=(kc == NKC - 1))

        def norm(pav, p, avn3):
            """normalize: avn[:, tc, p*64+d] = pav[:, tc, 1+d] / pav[:, tc, 0]"""
            ho = p * 64
            pavr = pav.rearrange("p (qc w) -> p qc w", w=128)
            den = denp.tile([128, 8], F32, name=f"den{id(pav)}_{p}", tag="den")
            den3 = den.rearrange("p (a b) -> p a b", b=1)
            nc.vector.tensor_copy(den3, pavr[:, :, 0:1])
            rcp = rcpp.tile([128, 8], F32, name=f"rcp{id(pav)}_{p}", tag="rcp")
            nc.vector.reciprocal(rcp, den)
            if FUSED_NORM:
                rcp_b = bass.AP(tensor=rcp.tensor, offset=rcp.offset,
                                ap=[list(rcp.ap[0]), [1, 8], [0, 64]])
                nc.vector.tensor_mul(avn3[:, :, ho:ho + 64], pavr[:, :, 1:65], rcp_b)
            else:
                for qc in range(8):
                    nc.vector.tensor_scalar_mul(
                        avn3[:, qc:qc + 1, ho:ho + 64],
                        pavr[:, qc:qc + 1, 1:65], rcp[:, qc:qc + 1])

        def emit_T(hc, qh, avn):
            aT3 = aT[hc].rearrange("p (tc t) -> p tc t", t=128)
            if TRANSPOSE_3D:
                nc.sync.dma_start_transpose(aT3[:, qh * 8:(qh + 1) * 8, :], avn)
            else:
                for tcn in range(8):
                    nc.sync.dma_start_transpose(
                        aT[hc][:, qh * 1024 + tcn * 128:qh * 1024 + (tcn + 1) * 128],
                        avn[:, tcn * 128:(tcn + 1) * 128])

        def emit_S_E(h, qh, kc, kt, qt):
            ho = (h % 2) * 64
            s = psS.tile([128, 1024], F32, name=f"s{h}_{qh}_{kc}", tag="s")
            ksl = slice(kc * 128, (kc + 1) * 128)
            for qq in range(2):
                qsl = slice(qh * 1024 + qq * 512, qh * 1024 + (qq + 1) * 512)
                nc.tensor.matmul(
                    s[:, qq * 512:(qq + 1) * 512],
                    kt[ho:ho + 64, ksl], qt[ho:ho + 64, qsl],
                    start=True, stop=True)
            pool = ex1p if (h, qh) == (1, 0) else expool
            tag = "ex1" if (h, qh) == (1, 0) else "ex"
            ex = pool.tile([128, 1024], BF16, name=f"ex{h}_{qh}_{kc}", tag=tag)
            nc.scalar.activation(ex, s, EXP, bias=0.0, scale=float(SCALE))
            return ex

        # ======== fused wall: units (h0,qh0)+(h1,qh0) share one S/E stream ==
        # All v-chunks and the remaining pair-0 qk slots run here as filler;
        # h1's AV is deferred into unit (h0,qh1) so the Activation engine gets
        # two units of exp supply while PE chews through the projection wall.
        emit_qk_slot(0, 0, psS)
        emit_qk_slot(4, 0, psS)
        emit_qk_slot(0, 1, psS)
        emit_rope_flush(psS)
        wall_slots = {0: (4, 1), 2: (4, 2), 4: (0, 2), 6: (4, 3), 8: (0, 3)}
        pav0 = psAV.tile([128, 1024], F32, name="pav0", tag="pav")
        ex0s = [None] * NKC
        ex1s = [None] * NKC
        avn00 = avnp.tile([128, 1024], BF16, name="avn00", tag="avn")
        avn00_3 = avn00.rearrange("p (tc w) -> p tc w", w=128)
        for kc in range(NKC):
            if kc in wall_slots:
                emit_qk_slot(*wall_slots[kc], psS)
            elif kc == 10:
                emit_rope_flush(psS)
            ex0s[kc] = emit_S_E(0, 0, kc, qk[4], qk[0])
            ex1s[kc] = emit_S_E(1, 0, kc, qk[4], qk[0])
            emit_v(kc, psF)
            if kc >= 2:
                emit_av(pav0, ex0s[kc - 2], kc - 2, 0)
        emit_av(pav0, ex0s[14], 14, 0)
        emit_av(pav0, ex0s[15], 15, 0)
        dbg("dbg_ex0", ex0s[0])
        norm(pav0, 0, avn00_3)
        dbg("dbg_avn0", avn00)

        pav1_box = [None]

        def a1_item(lo, hi):
            def f():
                if pav1_box[0] is None:
                    pav1_box[0] = psAV.tile([128, 1024], F32, name="pav1", tag="pav")
                for kc2 in range(lo, hi):
                    emit_av(pav1_box[0], ex1s[kc2], kc2, 1)
            return f

        def norm1_item():
            norm(pav1_box[0], 1, avn00_3)
            emit_T(0, 0, avn00)

        # filler items per unit index (u = 4*hc + 2*qh + p)
        def qk_item(jc, tb):
            return lambda: emit_qk_slot(jc, tb, psF)

        flush = lambda: emit_rope_flush(psF)
        unit_fill = {u: [] for u in range(16)}
        unit_fill[2] = [a1_item(0, 8), a1_item(8, 16), norm1_item,
                        qk_item(1, 0), qk_item(1, 1), qk_item(5, 0)]
        unit_fill[3] = [qk_item(1, 2), qk_item(5, 1), qk_item(1, 3)]
        unit_fill[4] = [qk_item(5, 2), qk_item(5, 3), flush]
        unit_fill[5] = [qk_item(2, 0), qk_item(2, 1), qk_item(6, 0)]
        unit_fill[6] = [qk_item(2, 2), qk_item(6, 1), qk_item(2, 3)]
        unit_fill[7] = [qk_item(6, 2), qk_item(6, 3), flush]
        unit_fill[8] = [qk_item(3, 0), qk_item(3, 1), qk_item(7, 0)]
        unit_fill[9] = [qk_item(3, 2), qk_item(7, 1), qk_item(3, 3)]
        unit_fill[10] = [qk_item(7, 2), qk_item(7, 3), flush]
        unit_fill[14] = [(lambda ec: (lambda: emit_proj(ec, 0, psF)))(ec) for ec in range(8)]
        unit_fill[15] = [(lambda ec: (lambda: emit_proj(ec, 1, psF)))(ec) for ec in range(8)]

        # ---------------- remaining attention units ----------------
        for hc in range(4):
            for qh in range(2):
                if hc == 0 and qh == 0:
                    continue  # handled by the fused wall above
                avn = avnp.tile([128, 1024], BF16, name=f"avn{hc}_{qh}", tag="avn")
                avn3 = avn.rearrange("p (tc w) -> p tc w", w=128)
                for p in range(2):
                    h = 2 * hc + p
                    u = 4 * hc + 2 * qh + p
                    qt = qk[hc]
                    kt = qk[4 + hc]
                    fills = unit_fill[u]
                    nfill = len(fills)
                    step = max(1, NKC // nfill) if nfill else NKC + 1
                    av_lag = 5 if u == 2 else 2
                    # pav is allocated at first use so psAV slot rotation
                    # follows emission order (pav1 is created inside u2's
                    # fillers, before this unit's first AV matmul).
                    pav = None
                    exs = [None] * NKC
                    fi = 0
                    for kc in range(NKC):
                        exs[kc] = emit_S_E(h, qh, kc, kt, qt)
                        if fi < nfill and kc % step == 0:
                            fills[fi]()
                            fi += 1
                        if kc >= av_lag:
                            if pav is None:
                                pav = psAV.tile([128, 1024], F32,
                                                name=f"pav{h}_{qh}", tag="pav")
                            emit_av(pav, exs[kc - av_lag], kc - av_lag, h)
                    while fi < nfill:
                        fills[fi]()
                        fi += 1
                    for kc in range(NKC - av_lag, NKC):
                        emit_av(pav, exs[kc], kc, h)
                    norm(pav, p, avn3)
                emit_T(hc, qh, avn)
        dbg("dbg_aT0", aT[0])

        # ---------------- tail: second half of projection ----------------
        # th2 prerun: open all 8 ec-groups and run their hcc 0-2 matmuls
        # while the last pair's normalize + transpose completes; the hcc3
        # matmul (stop) lands right after aT[3] arrives. Keeps PE busy with
        # no p-state reset across the transpose latency.
        tsl2 = slice(2 * 512, 3 * 512)
        tpools = [psS, psS, psF, psAV]
        t2 = [tpools[j].tile([128, 1024], F32, name=f"tt{j}",
                             tag="pav" if tpools[j] is psAV else "s")
              for j in range(4)]
        for j in range(4):
            for half in range(2):
                ec = 2 * j + half
                sl = slice(half * 512, (half + 1) * 512)
                for i in range(3):
                    nc.tensor.matmul(
                        t2[j][:, sl], wp_t[:, i * 1024 + ec * 128:i * 1024 + (ec + 1) * 128],
                        aT[i][:, tsl2], start=(i == 0), stop=False)
        for j in range(4):
            for half in range(2):
                ec = 2 * j + half
                sl = slice(half * 512, (half + 1) * 512)
                nc.tensor.matmul(
                    t2[j][:, sl], wp_t[:, 3 * 1024 + ec * 128:3 * 1024 + (ec + 1) * 128],
                    aT[3][:, tsl2], start=False, stop=True)
                osb = osbp.tile([128, 512], F32, name=f"osb{ec}_t2", tag="osb")
                nc.vector.tensor_scalar_add(osb, t2[j][:, sl], bpr_t[:, ec:ec + 1])
                nc.sync.dma_start(out[ec * 128:(ec + 1) * 128, tsl2], osb)
        th3_pools = [psS, psS, psF]
        for i, ec in enumerate(range(8)):
            emit_proj(ec, 3, th3_pools[i % 3])


def _input_specs():
    # name -> (shape, dtype_str)
    return {
        "xT": ((C, T), "bf16"), "wqkv": ((C, 3 * C // G), "bf16"),
        "wproj": ((C // G, C), "bf16"),
        "bqk": ((128, 8), "f32"), "bv": ((128, 520), "f32"),
        "bpr": ((128, 8), "f32"),
        "cosT": ((128, T), "bf16"), "sinT": ((128, T), "bf16"),
        "rope_perm": ((128, 128), "bf16"),
    }


def _build_program():
    import concourse.mybir as mybir
    import concourse.tile as tile
    from concourse import bacc

    nc = bacc.Bacc("TRN2", target_bir_lowering=False, debug=False)
    ins = {}
    for name, (shape, dts) in _input_specs().items():
        dt = mybir.dt.bfloat16 if dts == "bf16" else mybir.dt.float32
        ins[name] = nc.dram_tensor(name, list(shape), dt,
                                   kind="ExternalInput").ap()
    outs = {"out": nc.dram_tensor("out", [C, T], mybir.dt.float32,
                                  kind="ExternalOutput").ap()}
    with tile.TileContext(nc) as tc:
        _attn_body(tc, outs, ins)
    nc.compile()
    return nc


def _core_inputs(core, x, W_qkv, b_qkv, W_proj, b_proj, cosT, sinT, P):
    b, g = divmod(core, 2)
    f32 = np.float32
    bf16 = ml_dtypes.bfloat16
    xT = np.ascontiguousarray(np.asarray(x[b], dtype=f32).T).astype(bf16)
    W_qkv = np.asarray(W_qkv, dtype=f32)
    b_qkv = np.asarray(b_qkv, dtype=f32)
    q = W_qkv[:, g * 512:(g + 1) * 512]
    k = W_qkv[:, C + g * 512:C + (g + 1) * 512]
    v = W_qkv[:, 2 * C + g * 512:2 * C + (g + 1) * 512]
    wqkv = np.ascontiguousarray(np.concatenate([q, k, v], axis=1)).astype(bf16)
    bq = b_qkv[g * 512:(g + 1) * 512]
    bk = b_qkv[C + g * 512:C + (g + 1) * 512]
    bqk = np.ascontiguousarray(
        np.stack([bq[i * 128:(i + 1) * 128] for i in range(4)]
                 + [bk[i * 128:(i + 1) * 128] for i in range(4)], axis=1))
    bvr = b_qkv[2 * C + g * 512:2 * C + (g + 1) * 512].reshape(HG, 64)
    bvg = np.concatenate([np.ones((HG, 1), f32), bvr], axis=1).reshape(-1)  # (520,)
    bv = np.ascontiguousarray(np.tile(bvg[None, :], (128, 1)))
    wproj = np.ascontiguousarray(
        np.asarray(W_proj, dtype=f32)[g * 512:(g + 1) * 512]).astype(bf16)
    if g == 0:
        bpr = np.ascontiguousarray(
            np.asarray(b_proj, dtype=f32).reshape(8, 128).T)
    else:
        bpr = np.zeros((128, 8), dtype=f32)
    return {"xT": xT, "wqkv": wqkv, "wproj": wproj, "bqk": bqk, "bv": bv,
            "bpr": bpr, "cosT": cosT, "sinT": sinT, "rope_perm": P}


def run(x, W_qkv, b_qkv, W_proj, b_proj, trace=False):
    from concourse.bass_utils import run_bass_kernel_spmd

    if "nc" not in _CACHED:
        _CACHED["nc"] = _build_program()
    nc = _CACHED["nc"]

    bf16 = ml_dtypes.bfloat16
    cosT, sinT = _rope_tables()
    cosT = cosT.astype(bf16)
    sinT = sinT.astype(bf16)
    P = _perm_table().astype(bf16)
    in_maps = [_core_inputs(c, x, W_qkv, b_qkv, W_proj, b_proj, cosT, sinT, P)
               for c in range(8)]
    res = run_bass_kernel_spmd(nc, in_maps, core_ids=list(range(8)), trace=trace)
    parts = [np.asarray(r["out"], dtype=np.float32) for r in res.results]
    out = np.stack([(parts[2 * b] + parts[2 * b + 1]).T for b in range(B)], axis=0)
    return np.ascontiguousarray(out), res


def kernel(x, W_qkv, b_qkv, W_proj, b_proj):
    out, _ = run(x, W_qkv, b_qkv, W_proj, b_proj, trace=False)
    return out


# revision 31
# speedup vs baseline: 1.6074x; 1.0034x over previous
"""Multi-head attention (RoPE) Trainium2 Bass kernel — pipelined bf16 version.

Problem: B=4, T=2048, C=1024, H=16, d=64, fp32 in/out, full attention + RoPE.
Sharding: 8 cores = 4 batches x 2 head-groups (8 heads each). Each core
computes its batch's attention for its heads plus the partial (transposed)
output projection; the host sums the two head-group partials per batch and
transposes back.

Design notes (cost-model driven):
- All matmul operands are bf16 (1 cycle/row on PE, half the SBUF/DMA of f32).
- AV uses a token-major dataflow: out[q, 65] = ex_chunk^T @ [ones|v], using
  all 128 output partitions (halves AV PE time vs a 65-partition head-major
  form) and making softmax normalization a per-partition scalar multiply.
  The softmax denominator rides along as column 0 via the ones column of vg.
- Normalized attention output transposes back to head-dim-major via one
  SBUF->SBUF DMA xbar transpose per (head-pair, q-half); PE is not involved.
- One instruction stream software-pipelines everything: QKV chunk
  projections, v-projections and the second-half output projection run as PE
  filler inside the ACT-bound attention stretch so neither PE nor the
  Activation engine (exp) ever starves. RoPE for chunk i-1 is emitted inside
  chunk i's slot so its PE permutation-matmul never waits on DVE.
- Projection is emitted transposed (features on partitions) so its bias is a
  per-partition scalar; the host transposes the final result (untimed).
"""

import numpy as np
import ml_dtypes

B, T, C = 4, 2048, 1024
H, D = 16, 64
G = 2              # head groups (cores per batch)
HG = H // G        # heads per core = 8
CC = C // 128      # 8 contraction chunks
NKC = T // 128     # 16 key chunks
NTB = T // 512     # 4 t-blocks
ROPE_BASE = 10000.0
SCALE = 1.0 / np.sqrt(D)

FUSED_NORM = True      # stride-0 free-dim broadcast of 1/den in one DVE op
TRANSPOSE_3D = True    # one xbar DMA transpose per (pair, q-half)

_CACHED = {}


def _rope_tables():
    inv_freq = 1.0 / (ROPE_BASE ** (np.arange(0, D, 2, dtype=np.float32) / D))
    t = np.arange(T, dtype=np.float32)
    freqs = np.outer(t, inv_freq).astype(np.float32)          # (T, 32)
    emb = np.concatenate([freqs, freqs], axis=-1)             # (T, 64)
    cos = np.cos(emb).T.astype(np.float32)                    # (64, T)
    sin = np.sin(emb).T.astype(np.float32)                    # (64, T)
    cosT = np.concatenate([cos, cos], axis=0)                 # (128, T) two heads/chunk
    sinT = np.concatenate([sin, sin], axis=0)
    return np.ascontiguousarray(cosT), np.ascontiguousarray(sinT)


def _perm_table():
    # rot[d] = sum_s P[s, d] * raw[s] = rotate_half with sign, 2 heads/chunk
    P = np.zeros((128, 128), np.float32)
    for d in range(128):
        blk, dd = divmod(d, D)
        if dd < 32:
            P[blk * D + dd + 32, d] = -1.0
        else:
            P[blk * D + dd - 32, d] = 1.0
    return P


def _attn_body(tc, outs, ins):
    """Tile kernel body. ins/outs are dicts of DRAM APs."""
    import contextlib
    import concourse.bass as bass
    import concourse.mybir as mybir

    nc = tc.nc
    F32 = mybir.dt.float32
    BF16 = mybir.dt.bfloat16
    EXP = mybir.ActivationFunctionType.Exp

    xT = ins["xT"]            # (1024, 2048) bf16  x[b].T
    wqkv = ins["wqkv"]        # (1024, 1536) bf16  [Wq | Wk | Wv] cols for group
    wproj = ins["wproj"]      # (512, 1024) bf16
    bqk = ins["bqk"]          # (128, 8) f32 per-chunk per-partition bias
    bv = ins["bv"]            # (128, 520) f32 broadcast [1|v-bias] per head
    bpr = ins["bpr"]          # (128, 8) f32 proj bias (e-chunk cols; zeros g1)
    cosT_d = ins["cosT"]      # (128, 2048) bf16
    sinT_d = ins["sinT"]      # (128, 2048) bf16
    perm_d = ins["rope_perm"]  # (128, 128) bf16 signed rotate_half permutation
    out = outs["out"]         # (1024, 2048) f32 partial transposed output

    def dbg(name, tile_ap):
        if name in outs:
            nc.sync.dma_start(outs[name].bitcast(tile_ap.dtype), tile_ap)

    ctx = contextlib.ExitStack()
    with ctx:
        pers = ctx.enter_context(tc.tile_pool(name="pers", bufs=1))

        # ---------------- persistent tiles ----------------
        x_t = pers.tile([128, CC * T], BF16, name="x_t", tag="x_t")
        wqk_t = pers.tile([128, CC * 1024], BF16, name="wqk_t", tag="wqk_t")
        wv_t = pers.tile([128, CC * 512], BF16, name="wv_t", tag="wv_t")
        wp_t = pers.tile([128, 4 * 1024], BF16, name="wp_t", tag="wp_t")
        cos_t = pers.tile([128, T], BF16, name="cos_t", tag="cos_t")
        sin_t = pers.tile([128, T], BF16, name="sin_t", tag="sin_t")
        perm_t = pers.tile([128, 128], BF16, name="perm_t", tag="perm_t")
        bqk_t = pers.tile([128, 8], F32, name="bqk_t", tag="bqk_t")
        bv_t = pers.tile([128, 520], F32, name="bv_t", tag="bv_t")
        bpr_t = pers.tile([128, 8], F32, name="bpr_t", tag="bpr_t")
        qk = [pers.tile([128, T], BF16, name=f"qk{j}", tag=f"qk{j}") for j in range(8)]
        vg = [pers.tile([128, HG * 65], BF16, name=f"vg{k}", tag=f"vg{k}") for k in range(NKC)]
        aT = [pers.tile([128, T], BF16, name=f"aT{i}", tag=f"aT{i}") for i in range(4)]

        # ---------------- working pools ----------------
        # PSUM: psS 2x2 banks (scores/exp), psAV 1x2 banks (AV accum),
        # psF 1x2 banks (qkv/v/proj filler groups + rope perm outputs).
        psS = ctx.enter_context(tc.tile_pool(name="psS", bufs=2, space="PSUM"))
        psAV = ctx.enter_context(tc.tile_pool(name="psAV", bufs=1, space="PSUM"))
        psF = ctx.enter_context(tc.tile_pool(name="psF", bufs=1, space="PSUM"))
        expool = ctx.enter_context(tc.tile_pool(name="expool", bufs=6))
        ex1p = ctx.enter_context(tc.tile_pool(name="ex1p", bufs=16))
        rawp = ctx.enter_context(tc.tile_pool(name="rawp", bufs=2))
        tmpp = ctx.enter_context(tc.tile_pool(name="tmpp", bufs=2))
        tmpcp = ctx.enter_context(tc.tile_pool(name="tmpcp", bufs=2))
        denp = ctx.enter_context(tc.tile_pool(name="denp", bufs=2))
        rcpp = ctx.enter_context(tc.tile_pool(name="rcpp", bufs=2))
        avnp = ctx.enter_context(tc.tile_pool(name="avnp", bufs=2))
        osbp = ctx.enter_context(tc.tile_pool(name="osbp", bufs=5))

        # ---------------- input DMAs (no waits; ordered for earliest use) ---
        x3d = x_t.rearrange("p (c t) -> p c t", t=T)
        xTd = xT.rearrange("(c p) t -> p c t", p=128)
        wqk3 = wqk_t.rearrange("p (c e) -> p c e", e=1024)

        def w_slice(jc):
            # per-chunk 128-col slice of [Wq|Wk] for q/k chunk jc
            col0 = (jc % 4) * 128 + (512 if jc >= 4 else 0)
            nc.sync.dma_start(
                wqk3[:, :, col0:col0 + 128],
                wqkv[:, col0:col0 + 128].rearrange("(c p) e -> p c e", p=128))

        nc.sync.dma_start(perm_t, perm_d)
        w_slice(0)
        nc.sync.dma_start(x3d[:, :, 0:512], xTd[:, :, 0:512])
        nc.sync.dma_start(bqk_t, bqk)
        nc.sync.dma_start(cos_t[:, 0:1024], cosT_d[:, 0:1024])
        nc.sync.dma_start(sin_t[:, 0:1024], sinT_d[:, 0:1024])
        w_slice(4)
        nc.sync.dma_start(x3d[:, :, 512:1024], xTd[:, :, 512:1024])
        nc.sync.dma_start(cos_t[:, 1024:2048], cosT_d[:, 1024:2048])
        nc.sync.dma_start(sin_t[:, 1024:2048], sinT_d[:, 1024:2048])
        nc.sync.dma_start(wv_t.rearrange("p (c e) -> p c e", e=512),
                          wqkv[:, 1024:1536].rearrange("(c p) e -> p c e", p=128))
        nc.sync.dma_start(bv_t, bv)
        nc.sync.dma_start(x3d[:, :, 1024:1536], xTd[:, :, 1024:1536])
        nc.sync.dma_start(x3d[:, :, 1536:2048], xTd[:, :, 1536:2048])
        for jc in (1, 5, 2, 6, 3, 7):
            w_slice(jc)
        nc.sync.dma_start(bpr_t, bpr)
        nc.sync.dma_start(wp_t.rearrange("p (i e) -> p i e", e=1024),
                          wproj.rearrange("(i p) e -> p i e", p=128))

        # PE clock warmup: keep the tensor engine continuously busy from the
        # moment perm_t lands until the first real slot's inputs arrive, so
        # the p-state model reaches full clock before real work dispatches.
        warm = psF.tile([128, 1024], F32, name="warm", tag="s")
        for i in range(40):
            nc.tensor.matmul(warm[:, 0:128], perm_t, perm_t, start=True, stop=True)

        # ---------------- emitters ----------------
        pend = {"rope": None}  # (jc, tb, raw) awaiting perm-matmul + combine

        def emit_rope(ps_half):
            """Emit pending RoPE combine: perm-matmul into ps_half (psum
            (128,512) f32 slice), then DVE combine into qk[jc]."""
            jc, tb, raw = pend["rope"]
            pend["rope"] = None
            tsl = slice(tb * 512, (tb + 1) * 512)
            nc.tensor.matmul(ps_half, perm_t, raw, start=True, stop=True)
            tmp = tmpp.tile([128, 512], F32, name=f"tm{jc}_{tb}", tag="tmp")
            nc.vector.tensor_mul(tmp, ps_half, sin_t[:, tsl])
            tmpc = tmpcp.tile([128, 512], F32, name=f"tc{jc}_{tb}", tag="tmpc")
            nc.vector.tensor_mul(tmpc, raw, cos_t[:, tsl])
            nc.vector.tensor_add(qk[jc][:, tsl], tmp, tmpc)

        def emit_qk_slot(jc, tb, pool):
            """8 projection matmuls for q/k chunk jc, t-block tb, plus the
            RoPE combine of the previously emitted chunk."""
            col0 = (jc % 4) * 128 + (512 if jc >= 4 else 0)
            tsl = slice(tb * 512, (tb + 1) * 512)
            ps = pool.tile([128, 1024], F32, name=f"psq{jc}_{tb}", tag="s")
            if pend["rope"] is not None:
                emit_rope(ps[:, 512:1024])
            for c in range(CC):
                nc.tensor.matmul(
                    ps[:, 0:512], wqk_t[:, c * 1024 + col0:c * 1024 + col0 + 128],
                    x_t[:, c * T + tb * 512:c * T + (tb + 1) * 512],
                    start=(c == 0), stop=(c == CC - 1))
            raw = rawp.tile([128, 512], BF16, name=f"raw{jc}_{tb}", tag="raw")
            nc.vector.tensor_scalar_add(raw, ps[:, 0:512], bqk_t[:, jc:jc + 1])
            pend["rope"] = (jc, tb, raw)

        def emit_rope_flush(pool):
            ps = pool.tile([128, 1024], F32, name="psflush", tag="s")
            emit_rope(ps[:, 512:1024])

        def emit_v(kc, pool):
            """v for token chunk kc -> vg[kc] = [1|v] per head, bf16."""
            ps = pool.tile([128, 1024], F32, name=f"psv{kc}", tag="s")
            for c in range(CC):
                nc.tensor.matmul(
                    ps[:, 0:512], x_t[:, c * T + kc * 128:c * T + (kc + 1) * 128],
                    wv_t[:, c * 512:(c + 1) * 512],
                    start=(c == 0), stop=(c == CC - 1))
            vv = vg[kc].rearrange("p (g w) -> p g w", w=65)
            bvv = bv_t.rearrange("p (g w) -> p g w", w=65)
            psg = ps[:, 0:512].rearrange("p (g d) -> p g d", d=64)
            nc.vector.tensor_add(vv[:, :, 1:65], psg, bvv[:, :, 1:65])
            nc.vector.tensor_copy(vv[:, :, 0:1], bvv[:, :, 0:1])

        def emit_proj(ec, th, pool):
            """transposed proj: out rows = e-chunk ec, cols = tok block th."""
            tsl = slice(th * 512, (th + 1) * 512)
            ps = pool.tile([128, 1024], F32, name=f"psp{ec}_{th}", tag="s")
            for i in range(4):
                nc.tensor.matmul(
                    ps[:, 0:512], wp_t[:, i * 1024 + ec * 128:i * 1024 + (ec + 1) * 128],
                    aT[i][:, tsl], start=(i == 0), stop=(i == 3))
            osb = osbp.tile([128, 512], F32, name=f"osb{ec}_{th}", tag="osb")
            nc.vector.tensor_scalar_add(osb, ps[:, 0:512], bpr_t[:, ec:ec + 1])
            nc.sync.dma_start(out[ec * 128:(ec + 1) * 128, tsl], osb)

        def emit_av(pav, ex, kc, h):
            # start=True zeroes the whole PSUM bank, so only the first group
            # of each bank (qc 0 and 4) may set it; the bank-wide zero covers
            # the other interleaved accumulation groups' regions.
            mv = vg[kc][:, h * 65:(h + 1) * 65]
            for qc in range(8):
                nc.tensor.matmul(
                    pav[:, qc * 128:qc * 128 + 65],
                    ex[:, qc * 128:(qc + 1) * 128], mv,
                    start=(kc == 0 and qc % 4 == 0), stop=(kc == NKC - 1))

        def norm(pav, p, avn3, q0=0, q1=8):
            """normalize: avn[:, tc, p*64+d] = pav[:, tc, 1+d] / pav[:, tc, 0]"""
            ho = p * 64
            nq = q1 - q0
            pavr = pav.rearrange("p (qc w) -> p qc w", w=128)[:, q0:q1, :]
            den = denp.tile([128, 8], F32, name=f"den{id(pav)}_{p}_{q0}", tag="den")
            den3 = den.rearrange("p (a b) -> p a b", b=1)[:, 0:nq, :]
            nc.vector.tensor_copy(den3, pavr[:, :, 0:1])
            rcp = rcpp.tile([128, 8], F32, name=f"rcp{id(pav)}_{p}_{q0}", tag="rcp")
            nc.vector.reciprocal(rcp[:, 0:nq], den[:, 0:nq])
            if FUSED_NORM:
                rcp_b = bass.AP(tensor=rcp.tensor, offset=rcp.offset,
                                ap=[list(rcp.ap[0]), [1, nq], [0, 64]])
                nc.vector.tensor_mul(avn3[:, q0:q1, ho:ho + 64], pavr[:, :, 1:65], rcp_b)
            else:
                for qc in range(nq):
                    nc.vector.tensor_scalar_mul(
                        avn3[:, q0 + qc:q0 + qc + 1, ho:ho + 64],
                        pavr[:, qc:qc + 1, 1:65], rcp[:, qc:qc + 1])

        def emit_T(hc, qh, avn):
            aT3 = aT[hc].rearrange("p (tc t) -> p tc t", t=128)
            if TRANSPOSE_3D:
                nc.sync.dma_start_transpose(aT3[:, qh * 8:(qh + 1) * 8, :], avn)
            else:
                for tcn in range(8):
                    nc.sync.dma_start_transpose(
                        aT[hc][:, qh * 1024 + tcn * 128:qh * 1024 + (tcn + 1) * 128],
                        avn[:, tcn * 128:(tcn + 1) * 128])

        def emit_S_E(h, qh, kc, kt, qt):
            ho = (h % 2) * 64
            s = psS.tile([128, 1024], F32, name=f"s{h}_{qh}_{kc}", tag="s")
            ksl = slice(kc * 128, (kc + 1) * 128)
            for qq in range(2):
                qsl = slice(qh * 1024 + qq * 512, qh * 1024 + (qq + 1) * 512)
                nc.tensor.matmul(
                    s[:, qq * 512:(qq + 1) * 512],
                    kt[ho:ho + 64, ksl], qt[ho:ho + 64, qsl],
                    start=True, stop=True)
            pool = ex1p if (h, qh) == (1, 0) else expool
            tag = "ex1" if (h, qh) == (1, 0) else "ex"
            ex = pool.tile([128, 1024], BF16, name=f"ex{h}_{qh}_{kc}", tag=tag)
            nc.scalar.activation(ex, s, EXP, bias=0.0, scale=float(SCALE))
            return ex

        # ======== fused wall: units (h0,qh0)+(h1,qh0) share one S/E stream ==
        # All v-chunks and the remaining pair-0 qk slots run here as filler;
        # h1's AV is deferred into unit (h0,qh1) so the Activation engine gets
        # two units of exp supply while PE chews through the projection wall.
        emit_qk_slot(0, 0, psS)
        emit_qk_slot(4, 0, psS)
        emit_qk_slot(0, 1, psS)
        emit_rope_flush(psS)
        wall_slots = {0: (4, 1), 2: (4, 2), 4: (0, 2), 6: (4, 3), 8: (0, 3)}
        pav0 = psAV.tile([128, 1024], F32, name="pav0", tag="pav")
        ex0s = [None] * NKC
        ex1s = [None] * NKC
        avn00 = avnp.tile([128, 1024], BF16, name="avn00", tag="avn")
        avn00_3 = avn00.rearrange("p (tc w) -> p tc w", w=128)
        for kc in range(NKC):
            if kc in wall_slots:
                emit_qk_slot(*wall_slots[kc], psS)
            elif kc == 10:
                emit_rope_flush(psS)
            ex0s[kc] = emit_S_E(0, 0, kc, qk[4], qk[0])
            ex1s[kc] = emit_S_E(1, 0, kc, qk[4], qk[0])
            emit_v(kc, psF)
            if kc >= 2:
                emit_av(pav0, ex0s[kc - 2], kc - 2, 0)
        emit_av(pav0, ex0s[14], 14, 0)
        emit_av(pav0, ex0s[15], 15, 0)
        dbg("dbg_ex0", ex0s[0])
        norm(pav0, 0, avn00_3)
        dbg("dbg_avn0", avn00)

        pav1_box = [None]

        def a1_item(lo, hi):
            def f():
                if pav1_box[0] is None:
                    pav1_box[0] = psAV.tile([128, 1024], F32, name="pav1", tag="pav")
                for kc2 in range(lo, hi):
                    emit_av(pav1_box[0], ex1s[kc2], kc2, 1)
            return f

        def norm1_item():
            norm(pav1_box[0], 1, avn00_3)
            emit_T(0, 0, avn00)

        # filler items per unit index (u = 4*hc + 2*qh + p)
        def qk_item(jc, tb):
            return lambda: emit_qk_slot(jc, tb, psF)

        flush = lambda: emit_rope_flush(psF)
        # per-unit fillers as (iteration, item) pairs; iterations chosen so
        # every RoPE combine (riding the next slot) lands >=2 iterations
        # before the first S that reads the roped chunk.
        unit_fill = {u: [] for u in range(16)}
        unit_fill[2] = [(0, a1_item(0, 8)), (2, a1_item(8, 16)), (4, norm1_item),
                        (8, qk_item(1, 0)), (12, qk_item(1, 1))]
        unit_fill[3] = [(0, qk_item(5, 0)), (5, qk_item(5, 1))]
        unit_fill[4] = [(0, qk_item(5, 2)), (4, qk_item(5, 3)), (8, flush),
                        (12, qk_item(1, 2))]
        unit_fill[5] = [(0, qk_item(1, 3)), (4, qk_item(2, 0)),
                        (8, qk_item(2, 1)), (12, qk_item(6, 0))]
        unit_fill[6] = [(0, qk_item(2, 2)), (5, qk_item(6, 1)), (10, qk_item(2, 3))]
        unit_fill[7] = [(0, qk_item(6, 2)), (5, qk_item(6, 3)), (10, flush)]
        unit_fill[8] = [(0, qk_item(3, 0)), (5, qk_item(3, 1)), (10, qk_item(7, 0))]
        unit_fill[9] = [(0, qk_item(3, 2)), (5, qk_item(7, 1)), (10, qk_item(3, 3))]
        unit_fill[10] = [(0, qk_item(7, 2)), (5, qk_item(7, 3)), (10, flush)]
        unit_fill[14] = [(2 * ec + 2, (lambda e: (lambda: emit_proj(e, 0, psF)))(ec))
                         for ec in range(7)] + [(NKC, lambda: emit_proj(7, 0, psF))]
        unit_fill[15] = [(2 * ec, (lambda e: (lambda: emit_proj(e, 1, psF)))(ec))
                         for ec in range(8)]

        # ---------------- remaining attention units ----------------
        for hc in range(4):
            for qh in range(2):
                if hc == 0 and qh == 0:
                    continue  # handled by the fused wall above
                avn = avnp.tile([128, 1024], BF16, name=f"avn{hc}_{qh}", tag="avn")
                avn3 = avn.rearrange("p (tc w) -> p tc w", w=128)
                for p in range(2):
                    h = 2 * hc + p
                    u = 4 * hc + 2 * qh + p
                    qt = qk[hc]
                    kt = qk[4 + hc]
                    fills = dict(unit_fill[u])
                    av_lag = 5 if u == 2 else 2
                    # pav is allocated at first use so psAV slot rotation
                    # follows emission order (pav1 is created inside u2's
                    # fillers, before this unit's first AV matmul).
                    pav = None
                    exs = [None] * NKC
                    for kc in range(NKC):
                        exs[kc] = emit_S_E(h, qh, kc, kt, qt)
                        if kc in fills:
                            fills.pop(kc)()
                        if kc >= av_lag:
                            if pav is None:
                                pav = psAV.tile([128, 1024], F32,
                                                name=f"pav{h}_{qh}", tag="pav")
                            emit_av(pav, exs[kc - av_lag], kc - av_lag, h)
                    for it in sorted(fills):
                        fills.pop(it)()
                    for kc in range(NKC - av_lag, NKC):
                        emit_av(pav, exs[kc], kc, h)
                    if (hc, qh, p) == (3, 1, 1):
                        # final unit: normalize + transpose in halves so the
                        # tail's th2 projection can start on the first half
                        norm(pav, p, avn3, 0, 4)
                        aT3f = aT[3].rearrange("p (tc t) -> p tc t", t=128)
                        nc.sync.dma_start_transpose(
                            aT3f[:, 8:12, :], avn[:, 0:512])
                        norm(pav, p, avn3, 4, 8)
                        nc.sync.dma_start_transpose(
                            aT3f[:, 12:16, :], avn[:, 512:1024])
                    else:
                        norm(pav, p, avn3)
                if (hc, qh) != (3, 1):
                    emit_T(hc, qh, avn)
        dbg("dbg_aT0", aT[0])

        # ---------------- tail: second half of projection ----------------
        # th2 prerun: open all 8 ec-groups and run their hcc 0-2 matmuls
        # while the last pair's normalize + transpose completes; the hcc3
        # matmul (stop) lands right after aT[3] arrives. Keeps PE busy with
        # no p-state reset across the transpose latency.
        tsl2 = slice(2 * 512, 3 * 512)
        tpools = [psS, psS, psF, psAV]
        t2 = [tpools[j].tile([128, 1024], F32, name=f"tt{j}",
                             tag="pav" if tpools[j] is psAV else "s")
              for j in range(4)]
        for j in range(4):
            for half in range(2):
                ec = 2 * j + half
                sl = slice(half * 512, (half + 1) * 512)
                for i in range(3):
                    nc.tensor.matmul(
                        t2[j][:, sl], wp_t[:, i * 1024 + ec * 128:i * 1024 + (ec + 1) * 128],
                        aT[i][:, tsl2], start=(i == 0), stop=False)
        for j in range(4):
            for half in range(2):
                ec = 2 * j + half
                sl = slice(half * 512, (half + 1) * 512)
                nc.tensor.matmul(
                    t2[j][:, sl], wp_t[:, 3 * 1024 + ec * 128:3 * 1024 + (ec + 1) * 128],
                    aT[3][:, tsl2], start=False, stop=True)
                osb = osbp.tile([128, 512], F32, name=f"osb{ec}_t2", tag="osb")
                nc.vector.tensor_scalar_add(osb, t2[j][:, sl], bpr_t[:, ec:ec + 1])
                nc.sync.dma_start(out[ec * 128:(ec + 1) * 128, tsl2], osb)
        th3_pools = [psS, psS, psF]
        for i, ec in enumerate(range(8)):
            emit_proj(ec, 3, th3_pools[i % 3])


def _input_specs():
    # name -> (shape, dtype_str)
    return {
        "xT": ((C, T), "bf16"), "wqkv": ((C, 3 * C // G), "bf16"),
        "wproj": ((C // G, C), "bf16"),
        "bqk": ((128, 8), "f32"), "bv": ((128, 520), "f32"),
        "bpr": ((128, 8), "f32"),
        "cosT": ((128, T), "bf16"), "sinT": ((128, T), "bf16"),
        "rope_perm": ((128, 128), "bf16"),
    }


def _build_program():
    import concourse.mybir as mybir
    import concourse.tile as tile
    from concourse import bacc

    nc = bacc.Bacc("TRN2", target_bir_lowering=False, debug=False)
    ins = {}
    for name, (shape, dts) in _input_specs().items():
        dt = mybir.dt.bfloat16 if dts == "bf16" else mybir.dt.float32
        ins[name] = nc.dram_tensor(name, list(shape), dt,
                                   kind="ExternalInput").ap()
    outs = {"out": nc.dram_tensor("out", [C, T], mybir.dt.float32,
                                  kind="ExternalOutput").ap()}
    with tile.TileContext(nc) as tc:
        _attn_body(tc, outs, ins)
    nc.compile()
    return nc


def _core_inputs(core, x, W_qkv, b_qkv, W_proj, b_proj, cosT, sinT, P):
    b, g = divmod(core, 2)
    f32 = np.float32
    bf16 = ml_dtypes.bfloat16
    xT = np.ascontiguousarray(np.asarray(x[b], dtype=f32).T).astype(bf16)
    W_qkv = np.asarray(W_qkv, dtype=f32)
    b_qkv = np.asarray(b_qkv, dtype=f32)
    q = W_qkv[:, g * 512:(g + 1) * 512]
    k = W_qkv[:, C + g * 512:C + (g + 1) * 512]
    v = W_qkv[:, 2 * C + g * 512:2 * C + (g + 1) * 512]
    wqkv = np.ascontiguousarray(np.concatenate([q, k, v], axis=1)).astype(bf16)
    bq = b_qkv[g * 512:(g + 1) * 512]
    bk = b_qkv[C + g * 512:C + (g + 1) * 512]
    bqk = np.ascontiguousarray(
        np.stack([bq[i * 128:(i + 1) * 128] for i in range(4)]
                 + [bk[i * 128:(i + 1) * 128] for i in range(4)], axis=1))
    bvr = b_qkv[2 * C + g * 512:2 * C + (g + 1) * 512].reshape(HG, 64)
    bvg = np.concatenate([np.ones((HG, 1), f32), bvr], axis=1).reshape(-1)  # (520,)
    bv = np.ascontiguousarray(np.tile(bvg[None, :], (128, 1)))
    wproj = np.ascontiguousarray(
        np.asarray(W_proj, dtype=f32)[g * 512:(g + 1) * 512]).astype(bf16)
    if g == 0:
        bpr = np.ascontiguousarray(
            np.asarray(b_proj, dtype=f32).reshape(8, 128).T)
    else:
        bpr = np.zeros((128, 8), dtype=f32)
    return {"xT": xT, "wqkv": wqkv, "wproj": wproj, "bqk": bqk, "bv": bv,
            "bpr": bpr, "cosT": cosT, "sinT": sinT, "rope_perm": P}


def run(x, W_qkv, b_qkv, W_proj, b_proj, trace=False):
    from concourse.bass_utils import run_bass_kernel_spmd

    if "nc" not in _CACHED:
        _CACHED["nc"] = _build_program()
    nc = _CACHED["nc"]

    bf16 = ml_dtypes.bfloat16
    cosT, sinT = _rope_tables()
    cosT = cosT.astype(bf16)
    sinT = sinT.astype(bf16)
    P = _perm_table().astype(bf16)
    in_maps = [_core_inputs(c, x, W_qkv, b_qkv, W_proj, b_proj, cosT, sinT, P)
               for c in range(8)]
    res = run_bass_kernel_spmd(nc, in_maps, core_ids=list(range(8)), trace=trace)
    parts = [np.asarray(r["out"], dtype=np.float32) for r in res.results]
    out = np.stack([(parts[2 * b] + parts[2 * b + 1]).T for b in range(B)], axis=0)
    return np.ascontiguousarray(out), res


def kernel(x, W_qkv, b_qkv, W_proj, b_proj):
    out, _ = run(x, W_qkv, b_qkv, W_proj, b_proj, trace=False)
    return out
